# revision 42
# baseline (speedup 1.0000x reference)
"""BiCutLoss Trainium2 kernel (nn_BiCutLoss_52312701665760).

Reference computation (per batch row i of output[B, L, 2], labels[B, L]):
  temp = argmax(output, -1)            # 1 iff out1 > out0
  cut  = L if all(temp == 1) else (index of last 0 in temp)
  mask = arange(L) < cut
  r1   = where(labels == 1, -3.6/log2(j+2), 0.065)
  loss = sum(out1 * mask * r1) / B

Kernel formulation (exactly equivalent):
  d[j] = out0[j] - out1[j]                       # temp[j]==0  <=>  d[j] >= 0
  M[j] = max(d[j:], -1)  (reverse cummax; M[L] = -1 pad)
  thr  = 0 if M[0] >= 0 else -BIG                # all-ones row => mask all 1
  mask[j] = (M[j+1] >= thr)
  r1   = C + lab*preD   with C = 0.065, preD[j] = -3.6/log2(j+2) - C
  S_i  = sum_j mask*t1          A_i = sum_j mask*t1*lab*preD
  loss_i = C*S_i + A_i

Sharding: pure data parallel - B=4096 rows split as 512 rows x 8 cores; each
core computes per-row partials [128,1] (4 row-tiles of 128 partitions), host
sums and divides by B.

Per-core HBM traffic: out 16.8 MB + lab 8.4 MB = 25.2 MB; measured DMA-only
floor (vz probe) ~71.5-73 us/iter. Winning variant "vj" (~91.7 us/iter,
measured via interleaved repeat-delta at R=129):
  - ot [128, 8192] f32 DMA alternates SP/ACT HWDGE queues per tile
  - lab via gpsimd SWDGE cast-DMA i32->f16 (no ACT cast op)
  - d = t0 - t1 split: low 3/8 on Pool (Q7 f32-strided sub ~4.8us/full
    tile), high 5/8 on DVE, so neither engine stalls the scan
  - DVE: reverse scan max (~8.2us, the one expensive DVE op), thr (tiny),
    w = (M[1:] >= thr)*t1 (STT 1x, accum -> S_k), z = lab_b*preD (TT f16
    2x), w *= z (TT f16 2x)
  - ACT: sink Copy(w) accum -> A_k

Measured dead ends (same-process interleaved races): all-DMA-on-one-queue
(v0, +5us; each engine's HWDGE already spreads over 8 HW queues), SWDGE
vs HWDGE lab (±1us), io bufs=3 (±0), column-split DMA (±0), sub all-DVE
(+3us), scan-free iota-max formulation (vn, +11us), z/sub on Pool in f16
(vo, +29us: the Q7's f16 multiply is far slower than its f32 subtract),
in-place tensor_tensor_reduce (crashes the mesh).
"""

import os
from contextlib import ExitStack

import numpy as np

B, L = 4096, 4096
N_CORES = 8
ROWS_PER_CORE = B // N_CORES          # 512
P = 128                               # partitions per tile
TILES = ROWS_PER_CORE // P            # 4
C_CONST = 0.65 * 0.1                  # 0.065
BIG = 1e30

VARIANT = "vj"                        # kernel() uses this one

_CACHE = {}


def _build_nc(repeat: int = 1, variant: str = VARIANT):
    import concourse.mybir as mybir
    import concourse.tile as tile
    from concourse import bacc

    f32 = mybir.dt.float32
    f16 = mybir.dt.float16
    i32 = mybir.dt.int32
    Op = mybir.AluOpType
    Act = mybir.ActivationFunctionType

    # variant knobs
    #   ot_split: how ot's 4.19 MB/tile is routed over the two HWDGE queues
    #   lab_path: lab DMA queue + where the i32->f16 cast happens
    #   io_bufs:  io pool depth (DMA pipelining)
    #   use_ttr:  fuse w*z and the A-reduction into one DVE TTR (no ACT sink)
    knobs = {
        #         ot_split  lab_path  io_bufs  use_ttr  sub_eng
        "v0": ("sync",    "sync",   2, False, "pool"),
        "va": ("alt",     "alt",    2, False, "pool"),
        "vb": ("alt",     "swdge",  2, False, "pool"),
        "vd": ("col",     "swdge",  2, False, "pool"),
        "ve": ("alt",     "swdge",  3, False, "pool"),
        "vf": ("alt",     "swdge",  2, True,  "pool"),
        "vg": ("alt",     "swdge",  2, False, "act"),
        "vh": ("alt",     "swdge",  2, False, "split"),
        "vi": ("alt",     "alt",    2, False, "act"),
        "vj": ("alt",     "swdge",  2, False, "psplit"),
        "vk": ("alt",     "swdge",  2, False, "dve"),
        "vl": ("alt",     "alt",    2, False, "dve"),
        # vm = vj + lean: persistent M pad (no per-tile memset) and raw
        # acc_S/acc_A output with the C*S+A tail computed on host.
        "vm": ("alt",     "swdge",  2, False, "psplit"),
        # vn = scan-free: cut via m = max((j+1)*(d>=0)) fused in one STT;
        # d and z on Pool. Kills the 8.2us scan + memset + thr.
        "vn": ("alt",     "swdge",  2, False, "pool"),
        # vo = scan path, but sub AND z both on Pool: DVE keeps only
        # scan + STT + wz (+ tiny), ~14.6us/tile.
        "vo": ("alt",     "swdge",  2, False, "pool"),
    }
    probes = ("vz", "vy", "pscan", "psub", "pstt", "ptt", "pred", "ppool",
              "pact")
    if variant in probes:
        ot_split, lab_path, io_bufs, use_ttr, sub_eng = (
            "alt", "swdge", 2, False, "pool")
    else:
        ot_split, lab_path, io_bufs, use_ttr, sub_eng = knobs[variant]

    # Bacc (not raw Bass): its compile() runs generate_event_semaphores,
    # which splits multi-sem waits into standalone EventSemaphore
    # instructions (HW allows at most 1 wait per compute instruction).
    nc = bacc.Bacc("TRN2", target_bir_lowering=False, debug=False)

    out_d = nc.dram_tensor("out", [ROWS_PER_CORE, L * 2], f32, kind="ExternalInput")
    lab_d = nc.dram_tensor("lab", [ROWS_PER_CORE, L], i32, kind="ExternalInput")
    # pre holds two [P, L] constant planes: [:, 0:L] = preD, [:, L:2L] = j+1
    pre_d = nc.dram_tensor("pre", [P, L * 2], f32, kind="ExternalInput")
    res_d = nc.dram_tensor("res", [P, 1], f32, kind="ExternalOutput")

    out_t = out_d[:].rearrange("(n p) m -> n p m", p=P)   # [4, 128, 8192]
    lab_t = lab_d[:].rearrange("(n p) m -> n p m", p=P)   # [4, 128, 4096]

    with tile.TileContext(nc) as tc, ExitStack() as ctx:
        io_pool = ctx.enter_context(tc.tile_pool(name="io", bufs=io_bufs))
        pre_pool = ctx.enter_context(tc.tile_pool(name="pre", bufs=1))
        d_pool = ctx.enter_context(tc.tile_pool(name="d", bufs=2))
        m_pool = ctx.enter_context(tc.tile_pool(name="m", bufs=2))
        w_pool = ctx.enter_context(tc.tile_pool(name="w", bufs=2))
        z_pool = ctx.enter_context(tc.tile_pool(name="z", bufs=2))
        acc_pool = ctx.enter_context(tc.tile_pool(name="acc", bufs=2))

        done = False
        # preamble (outside the repeat loop; excluded from per-iter time):
        # preD as f16 via SWDGE cast DMA, one-time.
        pre_b = pre_pool.tile([P, L], f16)
        if variant not in ("pscan", "psub", "pstt", "ptt", "pred", "ppool",
                           "pact"):
            nc.gpsimd.dma_start(pre_b[:], pre_d[:][:, 0:L])
        if variant == "vn":
            iota1 = pre_pool.tile([P, L], f32, tag="iota1")
            nc.sync.dma_start(iota1[:], pre_d[:][:, L:L * 2])

        if variant in ("pscan", "psub", "pstt", "ptt", "pred", "ppool",
                       "pact"):
            pre_f = pre_pool.tile([P, L], f32, tag="pref")
            nc.sync.dma_start(pre_f[:], pre_d[:][:, 0:L])
            nc.scalar.activation(pre_b[:], pre_f[:], Act.Copy)
            # Engine-primitive probes: one op per repeat-iteration on
            # resident SBUF data (loaded once in the preamble). The
            # repeat-R-vs-repeat-r delta gives the op's true per-call cost.
            # Result is garbage by design.
            # All scratch allocated ONCE (bufs=1 pool); the loop rewrites
            # the same tiles from a single engine, so iterations serialize
            # in program order with no semaphores (pure op-rate measure).
            ot = pre_pool.tile([P, L * 2], f32, tag="pot")
            nc.sync.dma_start(ot[:], out_t[0])
            # single-HWDGE-queue preamble: the SP drain can only carry 4
            # sem waits, so probes avoid SWDGE/ACT queues entirely.
            lt0 = pre_pool.tile([P, L], i32, tag="plt0")
            nc.sync.dma_start(lt0[:], lab_t[0])
            lab_b = pre_pool.tile([P, L], f16, tag="plabb")
            nc.scalar.activation(lab_b[:], lt0[:], Act.Copy)
            x3 = ot[:].rearrange("p (l c) -> p l c", c=2)
            t0, t1 = x3[:, :, 0], x3[:, :, 1]
            d0 = pre_pool.tile([P, L], f16, tag="pd0")
            nc.vector.tensor_tensor(d0[:], t0, t1, Op.subtract)
            thr0 = pre_pool.tile([P, 1], f32, tag="pthr0")
            nc.vector.memset(thr0[:], 0.0)
            M = pre_pool.tile([P, L + 1], f16, tag="pM")
            nc.vector.memset(M[:, L:L + 1], -1.0)
            d = pre_pool.tile([P, L], f16, tag="pd")
            acc = pre_pool.tile([P, 1], f32, tag="pacc")
            w = pre_pool.tile([P, L], f16, tag="pw")
            z = pre_pool.tile([P, L], f16, tag="pz")
            mx = pre_pool.tile([P, 1], f32, tag="pmx")
            mxh = pre_pool.tile([P, 1], f16, tag="pmxh")
            loss_t = pre_pool.tile([P, 1], f32, tag="ploss")
            nc.vector.memset(loss_t[:], 0.0)
            for _r in range(repeat):
                if variant == "pscan":
                    nc.vector.tensor_tensor_scan(
                        M[:, 0:L][:, ::-1], d0[:, ::-1], d0[:, ::-1], -1.0,
                        Op.max, Op.max,
                    )
                    tgt = M
                elif variant == "psub":
                    nc.gpsimd.tensor_tensor(d[:], t0, t1, Op.subtract)
                    tgt = d
                elif variant == "pstt":
                    nc.vector.scalar_tensor_tensor(
                        w[:], d0[:], thr0[:], t1,
                        Op.is_ge, Op.mult,
                        accum_out=acc[:],
                    )
                    tgt = w
                elif variant == "ptt":
                    nc.vector.tensor_tensor(z[:], lab_b[:], pre_b[:], Op.mult)
                    tgt = z
                elif variant == "pred":
                    nc.vector.reduce_max(mx[:], d0[:], axis=mybir.AxisListType.X)
                    tgt = mx
                elif variant == "ppool":
                    # InstPool max on DVE (vs pred's tensor_reduce max)
                    nc.vector.pool_max(mxh[:], d0[:])
                    tgt = mxh
                else:  # pact: ACT sink copy with accum
                    nc.scalar.activation(
                        w[:], d0[:], Act.Copy,
                        accum_out=acc[:],
                    )
                    tgt = w
                # cheap consumer: keeps every write consumed so the final
                # Drain doesn't accumulate unbounded sem waits.
                nc.vector.tensor_tensor(
                    loss_t[:], loss_t[:], tgt[:, 0:1], Op.add)
            nc.sync.dma_start(res_d[:], loss_t[:])
            done = True

        if variant in ("vz", "vy"):
            # DMA-only probe: measures the pure HBM streaming floor.
            # vz: balanced SP/ACT HWDGE queues (12.6 MB each); vy: all on
            # SP. No SWDGE (the SP drain allows at most 4 sem waits).
            # Result is garbage (zeros) by design.
            loss_t = acc_pool.tile([P, 1], f32, tag="loss")
            nc.vector.memset(loss_t[:], 0.0)
            for _r in range(repeat):
                for k in range(TILES):
                    ot = io_pool.tile([P, L * 2], f32, tag="ot")
                    lt = io_pool.tile([P, L], i32, tag="lt")
                    if variant == "vy":
                        nc.sync.dma_start(ot[:], out_t[k])
                        nc.sync.dma_start(lt[:], lab_t[k])
                    else:
                        ot_e = nc.sync if k % 2 == 0 else nc.scalar
                        lab_e = nc.scalar if k % 2 == 0 else nc.sync
                        ot_e.dma_start(ot[:], out_t[k])
                        lab_e.dma_start(lt[:], lab_t[k])
            nc.sync.dma_start(res_d[:], loss_t[:])
            done = True

        for _r in (range(repeat) if not done else ()):
            acc_S = acc_pool.tile([P, TILES], f32, tag="accS")
            acc_A = acc_pool.tile([P, TILES], f32, tag="accA")
            for k in range(TILES):
                ot = io_pool.tile([P, L * 2], f32, tag="ot")
                if ot_split == "sync":
                    nc.sync.dma_start(ot[:], out_t[k])
                elif ot_split == "alt":
                    eng = nc.sync if k % 2 == 0 else nc.scalar
                    eng.dma_start(ot[:], out_t[k])
                else:  # col: half the columns per HWDGE queue, every tile
                    nc.sync.dma_start(ot[:, 0:L], out_t[k][:, 0:L])
                    nc.scalar.dma_start(ot[:, L:L * 2], out_t[k][:, L:L * 2])

                if lab_path == "swdge":
                    # SWDGE cast DMA: reads i32 from HBM, writes f16 SBUF.
                    lab_b = z_pool.tile([P, L], f16, tag="labb")
                    nc.gpsimd.dma_start(lab_b[:], lab_t[k])
                else:
                    lab_eng = nc.sync if (
                        lab_path == "sync" or k % 2 == 1) else nc.scalar
                    lt = io_pool.tile([P, L], i32, tag="lt")
                    lab_eng.dma_start(lt[:], lab_t[k])
                    # ACT: int32 -> f16 cast on-engine.
                    lab_b = z_pool.tile([P, L], f16, tag="labb")
                    nc.scalar.activation(lab_b[:], lt[:], Act.Copy)

                x3 = ot[:].rearrange("p (l c) -> p l c", c=2)
                t0 = x3[:, :, 0]
                t1 = x3[:, :, 1]

                if variant == "vn":
                    # scan-free mask: d = t0-t1 (Pool); y = (d>=0)*(j+1)
                    # in ONE DVE STT; m = max_j y = 1 + (last j with d>=0),
                    # or 0 if none; mask[j] = (j+1 < m'), m' = m (or L+1
                    # when m==0).  z = lab*preD on Pool (Q7 mult).
                    d = d_pool.tile([P, L], f16)
                    nc.gpsimd.tensor_tensor(d[:], t0, t1, Op.subtract)
                    # z on DVE (f16 2x) — the Q7's f16 multiply is slow
                    z = z_pool.tile([P, L], f16, tag="z")
                    nc.vector.tensor_tensor(z[:], lab_b[:], pre_b[:],
                                            Op.mult)
                    y = m_pool.tile([P, L], f32, tag="y")
                    nc.vector.scalar_tensor_tensor(
                        y[:], d[:], 0.0, iota1[:],
                        Op.is_ge, Op.mult,
                    )
                    mq = acc_pool.tile([P, 1], f32, tag="mq")
                    nc.vector.reduce_max(mq[:], y[:],
                                         axis=mybir.AxisListType.X)
                    # m' = m + (m==0)*(L+1), two tiny [P,1] ops
                    eq2 = acc_pool.tile([P, 1], f32, tag="eq2")
                    nc.vector.tensor_scalar(
                        eq2[:], mq[:], 0.0, float(L + 1), Op.is_equal,
                        Op.mult)
                    nc.vector.tensor_tensor(mq[:], mq[:], eq2[:], Op.add)
                    # w = (iota1 < m') * t1, S_k = sum(w)
                    w = w_pool.tile([P, L], f16)
                    nc.vector.scalar_tensor_tensor(
                        w[:], iota1[:], mq[:], t1,
                        Op.is_lt, Op.mult,
                        accum_out=acc_S[:, k:k + 1],
                    )
                    # w *= z (TT f16, 2x, in-place)
                    nc.vector.tensor_tensor(w[:], w[:], z[:], Op.mult)
                    # ACT: sink copy with accumulator -> A_k
                    sink = w_pool.tile([P, L], f16, tag="sink")
                    nc.scalar.activation(
                        sink[:], w[:], Act.Copy,
                        accum_out=acc_A[:, k:k + 1],
                    )
                    continue

                # d = t0 - t1 (f16 out).  Pool's Q7 software TT is slow on
                # strided f32 reads, so variants move this to ACT.
                d = d_pool.tile([P, L], f16)
                if sub_eng == "pool":
                    nc.gpsimd.tensor_tensor(d[:], t0, t1, Op.subtract)
                elif sub_eng == "dve":
                    nc.vector.tensor_tensor(d[:], t0, t1, Op.subtract)
                elif sub_eng == "psplit":
                    # low 3/8 on Pool (its Q7 sub is ~4x slower per elem),
                    # high 5/8 on DVE, so both finish together.
                    h = 3 * L // 8
                    nc.gpsimd.tensor_tensor(
                        d[:, 0:h], t0[:, 0:h], t1[:, 0:h], Op.subtract)
                    nc.vector.tensor_tensor(
                        d[:, h:L], t0[:, h:L], t1[:, h:L], Op.subtract)
                elif sub_eng == "act":
                    nc.scalar.tensor_tensor(d[:], t0, t1, Op.subtract)
                else:  # split: low half on Pool, high half on ACT
                    h = L // 2
                    nc.gpsimd.tensor_tensor(
                        d[:, 0:h], t0[:, 0:h], t1[:, 0:h], Op.subtract)
                    nc.scalar.tensor_tensor(
                        d[:, h:L], t0[:, h:L], t1[:, h:L], Op.subtract)

                # DVE scan: M[j] = max(d[j:], -1), M[L] = -1 pad (f16).
                # Ordered before z so ACT's sink(k-1) hides under the
                # scan+STT window instead of stalling DVE's queue head.
                M = m_pool.tile([P, L + 1], f16)
                nc.vector.memset(M[:, L:L + 1], -1.0)
                nc.vector.tensor_tensor_scan(
                    M[:, 0:L][:, ::-1], d[:, ::-1], d[:, ::-1], -1.0,
                    Op.max, Op.max,
                )

                # tiny (DVE): thr = 0 if M[0] >= 0 else -BIG, one fused TS:
                # (M0 < 0) * -BIG
                thr = acc_pool.tile([P, 1], f32, tag="thr")
                nc.vector.tensor_scalar(
                    thr[:], M[:, 0:1], 0.0, -BIG, Op.is_lt, Op.mult
                )

                # DVE: w = (M[j+1] >= thr) * t1 (f16 out), S_k = sum(w).
                w = w_pool.tile([P, L], f16)
                nc.vector.scalar_tensor_tensor(
                    w[:], M[:, 1:L + 1], thr[:], t1,
                    Op.is_ge, Op.mult,
                    accum_out=acc_S[:, k:k + 1],
                )

                # z = lab_b * preD (TT f16): on Pool for vo (frees DVE),
                # else DVE (2x), late on purpose.
                z = z_pool.tile([P, L], f16, tag="z")
                z_eng = nc.gpsimd if variant == "vo" else nc.vector
                z_eng.tensor_tensor(z[:], lab_b[:], pre_b[:], Op.mult)

                if use_ttr:
                    # DVE: w = w*z fused with A_k = sum(w*z); no ACT sink.
                    nc.vector.tensor_tensor_reduce(
                        w[:], w[:], z[:], 1.0, 0.0, Op.mult, Op.add,
                        accum_out=acc_A[:, k:k + 1],
                    )
                else:
                    # DVE: w *= z (TT f16, 2x, in-place).
                    nc.vector.tensor_tensor(w[:], w[:], z[:], Op.mult)

                    # ACT: sink copy with accumulator -> A_k = sum(w*z).
                    sink = w_pool.tile([P, L], f16, tag="sink")
                    nc.scalar.activation(
                        sink[:], w[:], Act.Copy,
                        accum_out=acc_A[:, k:k + 1],
                    )

            if k == TILES - 1:
                # tail: loss_i = C*sum_k S_k + sum_k A_k
                t4 = acc_pool.tile([P, TILES], f32, tag="t4")
                nc.vector.tensor_scalar(t4[:], acc_S[:], C_CONST, None, Op.mult)
                nc.vector.tensor_tensor(t4[:], t4[:], acc_A[:], Op.add)
                loss_t = acc_pool.tile([P, 1], f32, tag="loss")
                nc.vector.reduce_sum(loss_t[:], t4[:], axis=mybir.AxisListType.X)

        if not done:
            nc.sync.dma_start(res_d[:], loss_t[:])

    nc.compile()
    return nc


def _pre_tile() -> np.ndarray:
    j = np.arange(L, dtype=np.float64)
    pre2 = (-3.6 / np.log2(j + 2.0) - C_CONST).astype(np.float32)
    iota1 = (j + 1.0).astype(np.float32)
    plane = np.concatenate([pre2, iota1])
    return np.ascontiguousarray(np.tile(plane[None, :], (P, 1)))


def _get_nc(repeat: int = 1, variant: str = VARIANT):
    key = (repeat, variant)
    if key not in _CACHE:
        _CACHE[key] = _build_nc(repeat=repeat, variant=variant)
    return _CACHE[key]


def make_in_maps(output: np.ndarray, labels: np.ndarray):
    pre = _pre_tile()
    in_maps = []
    for c in range(N_CORES):
        sl = slice(c * ROWS_PER_CORE, (c + 1) * ROWS_PER_CORE)
        in_maps.append({
            "out": np.ascontiguousarray(output[sl]).reshape(ROWS_PER_CORE, L * 2),
            "lab": np.ascontiguousarray(labels[sl]),
            "pre": pre,
        })
    return in_maps


def kernel(output: np.ndarray, labels: np.ndarray) -> np.ndarray:
    from concourse.bass_utils import run_bass_kernel_spmd

    nc = _get_nc(repeat=1)
    in_maps = make_in_maps(output, labels)
    r = run_bass_kernel_spmd(nc, in_maps, core_ids=list(range(N_CORES)))
    total = 0.0
    for res in r.results:
        total += float(res["res"].astype(np.float64).sum())
    return np.float32(total / B)


if __name__ == "__main__":
    # quick standalone run (full inputs, random)
    rng = np.random.default_rng(0)
    out = rng.standard_normal((B, L, 2)).astype(np.float32)
    lab = rng.integers(0, 2, size=(B, L)).astype(np.int32)
    print("loss:", kernel(out, lab))


# revision 63
# speedup vs baseline: 1.0724x; 1.0724x over previous
"""BiCutLoss Trainium2 kernel (nn_BiCutLoss_52312701665760).

Reference computation (per batch row i of output[B, L, 2], labels[B, L]):
  temp = argmax(output, -1)            # 1 iff out1 > out0
  cut  = L if all(temp == 1) else (index of last 0 in temp)
  mask = arange(L) < cut
  r1   = where(labels == 1, -3.6/log2(j+2), 0.065)
  loss = sum(out1 * mask * r1) / B

Kernel formulation (exactly equivalent):
  d[j] = out0[j] - out1[j]                       # temp[j]==0  <=>  d[j] >= 0
  M[j] = max(d[j:], -1)  (reverse cummax; M[L] = -1 pad)
  thr  = 0 if M[0] >= 0 else -BIG                # all-ones row => mask all 1
  mask[j] = (M[j+1] >= thr)
  r1   = C + lab*preD   with C = 0.065, preD[j] = -3.6/log2(j+2) - C
  S_i  = sum_j mask*t1          A_i = sum_j mask*t1*lab*preD
  loss_i = C*S_i + A_i

Sharding: pure data parallel - B=4096 rows split as 512 rows x 8 cores; each
core computes per-row partials [128,1] (4 row-tiles of 128 partitions), host
sums and divides by B.

Per-core HBM traffic: out 16.8 MB + lab 8.4 MB = 25.2 MB; measured DMA-only
floor (vz/vz2 probes) ~71.5-73 us/iter. Winning variant "vr" (~85.3 us/iter
in a clean window, measured via interleaved repeat-delta at R=129 vs 33):
  - ot [128, 8192] f32 DMA alternates SP/ACT HWDGE queues per tile
  - lab via gpsimd SWDGE cast-DMA i32->f16 (no ACT cast op; cast-DMA is
    not slower than raw, vz2 probe)
  - ACT casts t0/t1 (strided f32 -> contiguous f16 t0h/t1h); d = t0h-t1h
    on DVE as a cheap f16 2x TT
  - DVE (all f16, ~13us/tile): reverse scan max (~8.2us, irreducible),
    thr (tiny), w = (M[1:] >= thr)*t1h (STT 2x, accum -> S_k),
    z = lab_b*preD (TT 2x), w *= z (TT 2x)
  - M pad column preset once in two persistent buffers (no per-tile
    memset); ACT: sink Copy(w) accum -> A_k

Measured dead ends (same-process interleaved races): all-DMA-on-one-queue
(v0, +5us; each engine's HWDGE already spreads over 8 HW queues), SWDGE
vs HWDGE lab (±1us), io bufs=3 (±0), column-split DMA (±0), sub all-DVE
(+3us) or all-Pool (+2..6us: Pool->DVE handoff latency), scan-free
iota-max formulation (vn, +11us), z/sub on Pool in f16 (vo, +29us: the
Q7's f16 multiply is far slower than its f32 subtract), in-place d/z +
io bufs=3 (vs, +36us: in-place WAR serialization), deferring the ACT
sink one tile (vt, ±0), tensor_tensor_reduce in any form including the
qr.py dummy-broadcast pattern (crashes the device mesh).
"""

import os
from contextlib import ExitStack

import numpy as np

B, L = 4096, 4096
N_CORES = 8
ROWS_PER_CORE = B // N_CORES          # 512
P = 128                               # partitions per tile
TILES = ROWS_PER_CORE // P            # 4
C_CONST = 0.65 * 0.1                  # 0.065
BIG = 1e30

VARIANT = "vr"                        # kernel() uses this one

_CACHE = {}


def _build_nc(repeat: int = 1, variant: str = VARIANT):
    import concourse.mybir as mybir
    import concourse.tile as tile
    from concourse import bacc

    f32 = mybir.dt.float32
    f16 = mybir.dt.float16
    i32 = mybir.dt.int32
    Op = mybir.AluOpType
    Act = mybir.ActivationFunctionType

    # variant knobs
    #   ot_split: how ot's 4.19 MB/tile is routed over the two HWDGE queues
    #   lab_path: lab DMA queue + where the i32->f16 cast happens
    #   io_bufs:  io pool depth (DMA pipelining)
    #   use_ttr:  fuse w*z and the A-reduction into one DVE TTR (no ACT sink)
    knobs = {
        #         ot_split  lab_path  io_bufs  use_ttr  sub_eng
        "v0": ("sync",    "sync",   2, False, "pool"),
        "va": ("alt",     "alt",    2, False, "pool"),
        "vb": ("alt",     "swdge",  2, False, "pool"),
        "vd": ("col",     "swdge",  2, False, "pool"),
        "ve": ("alt",     "swdge",  3, False, "pool"),
        "vf": ("alt",     "swdge",  2, True,  "pool"),
        "vg": ("alt",     "swdge",  2, False, "act"),
        "vh": ("alt",     "swdge",  2, False, "split"),
        "vi": ("alt",     "alt",    2, False, "act"),
        "vj": ("alt",     "swdge",  2, False, "psplit"),
        "vk": ("alt",     "swdge",  2, False, "dve"),
        "vl": ("alt",     "alt",    2, False, "dve"),
        # vm = vj + lean: persistent M pad (no per-tile memset) and raw
        # acc_S/acc_A output with the C*S+A tail computed on host.
        "vm": ("alt",     "swdge",  2, False, "psplit"),
        # vn = scan-free: cut via m = max((j+1)*(d>=0)) fused in one STT;
        # d and z on Pool. Kills the 8.2us scan + memset + thr.
        "vn": ("alt",     "swdge",  2, False, "pool"),
        # vo = scan path, but sub AND z both on Pool: DVE keeps only
        # scan + STT + wz (+ tiny), ~14.6us/tile.
        "vo": ("alt",     "swdge",  2, False, "pool"),
        # vp/vq = all-f16 DVE: ACT casts t1 (and t0 for vp) to contiguous
        # f16 so the STT gets 2x; sub on DVE-f16 (vp) or Pool-f32 (vq).
        "vp": ("alt",     "swdge",  2, False, "dve"),
        "vq": ("alt",     "swdge",  2, False, "pool"),
        # vr = vp + persistent M buffers (pad memset once, not per tile).
        "vr": ("alt",     "swdge",  2, False, "dve"),
        # vs = vr + in-place d (into t0h) and z (into lab_b) + io bufs=3.
        "vs": ("alt",     "swdge",  3, False, "dve"),
        # vt = vr + sink deferred one tile, so ACT's casts for tile k+1
        # aren't queued behind sink(k) (which waits on wz(k)).
        "vt": ("alt",     "swdge",  2, False, "dve"),
        # vu = vr + wz fused with the A-accum in one dummy-broadcast TTR
        # on DVE (qr.py pattern); ACT does only the two casts.
        "vu": ("alt",     "swdge",  2, False, "dve"),
    }
    probes = ("vz", "vy", "vz2", "pscan", "psub", "pstt", "ptt", "pred",
              "ppool", "pact")
    if variant in probes:
        ot_split, lab_path, io_bufs, use_ttr, sub_eng = (
            "alt", "swdge", 2, False, "pool")
    else:
        ot_split, lab_path, io_bufs, use_ttr, sub_eng = knobs[variant]

    # Bacc (not raw Bass): its compile() runs generate_event_semaphores,
    # which splits multi-sem waits into standalone EventSemaphore
    # instructions (HW allows at most 1 wait per compute instruction).
    nc = bacc.Bacc("TRN2", target_bir_lowering=False, debug=False)

    out_d = nc.dram_tensor("out", [ROWS_PER_CORE, L * 2], f32, kind="ExternalInput")
    lab_d = nc.dram_tensor("lab", [ROWS_PER_CORE, L], i32, kind="ExternalInput")
    # pre holds two [P, L] constant planes: [:, 0:L] = preD, [:, L:2L] = j+1
    pre_d = nc.dram_tensor("pre", [P, L * 2], f32, kind="ExternalInput")
    res_d = nc.dram_tensor("res", [P, 1], f32, kind="ExternalOutput")

    out_t = out_d[:].rearrange("(n p) m -> n p m", p=P)   # [4, 128, 8192]
    lab_t = lab_d[:].rearrange("(n p) m -> n p m", p=P)   # [4, 128, 4096]

    with tile.TileContext(nc) as tc, ExitStack() as ctx:
        io_pool = ctx.enter_context(tc.tile_pool(name="io", bufs=io_bufs))
        pre_pool = ctx.enter_context(tc.tile_pool(name="pre", bufs=1))
        d_pool = ctx.enter_context(tc.tile_pool(name="d", bufs=2))
        m_pool = ctx.enter_context(tc.tile_pool(name="m", bufs=2))
        w_pool = ctx.enter_context(tc.tile_pool(name="w", bufs=2))
        z_pool = ctx.enter_context(tc.tile_pool(name="z", bufs=2))
        acc_pool = ctx.enter_context(tc.tile_pool(name="acc", bufs=2))

        done = False
        # preamble (outside the repeat loop; excluded from per-iter time):
        # preD as f16 via SWDGE cast DMA, one-time.
        pre_b = pre_pool.tile([P, L], f16)
        if variant not in ("pscan", "psub", "pstt", "ptt", "pred", "ppool",
                           "pact"):
            nc.gpsimd.dma_start(pre_b[:], pre_d[:][:, 0:L])
        if variant == "vn":
            iota1 = pre_pool.tile([P, L], f32, tag="iota1")
            nc.sync.dma_start(iota1[:], pre_d[:][:, L:L * 2])
        if variant in ("vr", "vs", "vt", "vu"):
            M_a = pre_pool.tile([P, L + 1], f16, tag="Mpa")
            M_b = pre_pool.tile([P, L + 1], f16, tag="Mpb")
            M_pers = [M_a, M_b]
            nc.vector.memset(M_a[:, L:L + 1], -1.0)
            nc.vector.memset(M_b[:, L:L + 1], -1.0)

        if variant in ("pscan", "psub", "pstt", "ptt", "pred", "ppool",
                       "pact"):
            pre_f = pre_pool.tile([P, L], f32, tag="pref")
            nc.sync.dma_start(pre_f[:], pre_d[:][:, 0:L])
            nc.scalar.activation(pre_b[:], pre_f[:], Act.Copy)
            # Engine-primitive probes: one op per repeat-iteration on
            # resident SBUF data (loaded once in the preamble). The
            # repeat-R-vs-repeat-r delta gives the op's true per-call cost.
            # Result is garbage by design.
            # All scratch allocated ONCE (bufs=1 pool); the loop rewrites
            # the same tiles from a single engine, so iterations serialize
            # in program order with no semaphores (pure op-rate measure).
            ot = pre_pool.tile([P, L * 2], f32, tag="pot")
            nc.sync.dma_start(ot[:], out_t[0])
            # single-HWDGE-queue preamble: the SP drain can only carry 4
            # sem waits, so probes avoid SWDGE/ACT queues entirely.
            lt0 = pre_pool.tile([P, L], i32, tag="plt0")
            nc.sync.dma_start(lt0[:], lab_t[0])
            lab_b = pre_pool.tile([P, L], f16, tag="plabb")
            nc.scalar.activation(lab_b[:], lt0[:], Act.Copy)
            x3 = ot[:].rearrange("p (l c) -> p l c", c=2)
            t0, t1 = x3[:, :, 0], x3[:, :, 1]
            d0 = pre_pool.tile([P, L], f16, tag="pd0")
            nc.vector.tensor_tensor(d0[:], t0, t1, Op.subtract)
            thr0 = pre_pool.tile([P, 1], f32, tag="pthr0")
            nc.vector.memset(thr0[:], 0.0)
            M = pre_pool.tile([P, L + 1], f16, tag="pM")
            nc.vector.memset(M[:, L:L + 1], -1.0)
            d = pre_pool.tile([P, L], f16, tag="pd")
            acc = pre_pool.tile([P, 1], f32, tag="pacc")
            w = pre_pool.tile([P, L], f16, tag="pw")
            z = pre_pool.tile([P, L], f16, tag="pz")
            mx = pre_pool.tile([P, 1], f32, tag="pmx")
            mxh = pre_pool.tile([P, 1], f16, tag="pmxh")
            loss_t = pre_pool.tile([P, 1], f32, tag="ploss")
            nc.vector.memset(loss_t[:], 0.0)
            for _r in range(repeat):
                if variant == "pscan":
                    nc.vector.tensor_tensor_scan(
                        M[:, 0:L][:, ::-1], d0[:, ::-1], d0[:, ::-1], -1.0,
                        Op.max, Op.max,
                    )
                    tgt = M
                elif variant == "psub":
                    nc.gpsimd.tensor_tensor(d[:], t0, t1, Op.subtract)
                    tgt = d
                elif variant == "pstt":
                    nc.vector.scalar_tensor_tensor(
                        w[:], d0[:], thr0[:], t1,
                        Op.is_ge, Op.mult,
                        accum_out=acc[:],
                    )
                    tgt = w
                elif variant == "ptt":
                    nc.vector.tensor_tensor(z[:], lab_b[:], pre_b[:], Op.mult)
                    tgt = z
                elif variant == "pred":
                    nc.vector.reduce_max(mx[:], d0[:], axis=mybir.AxisListType.X)
                    tgt = mx
                elif variant == "ppool":
                    # InstPool max on DVE (vs pred's tensor_reduce max)
                    nc.vector.pool_max(mxh[:], d0[:])
                    tgt = mxh
                else:  # pact: ACT sink copy with accum
                    nc.scalar.activation(
                        w[:], d0[:], Act.Copy,
                        accum_out=acc[:],
                    )
                    tgt = w
                # cheap consumer: keeps every write consumed so the final
                # Drain doesn't accumulate unbounded sem waits.
                nc.vector.tensor_tensor(
                    loss_t[:], loss_t[:], tgt[:, 0:1], Op.add)
            nc.sync.dma_start(res_d[:], loss_t[:])
            done = True

        if variant in ("vz", "vy", "vz2"):
            # DMA-only probe: measures the pure HBM streaming floor.
            # vz: balanced SP/ACT HWDGE queues (12.6 MB each); vy: all on
            # SP; vz2: the vj layout (alt ot + SWDGE cast-DMA lab).
            # Result is garbage (zeros) by design.
            loss_t = acc_pool.tile([P, 1], f32, tag="loss")
            nc.vector.memset(loss_t[:], 0.0)
            for _r in range(repeat):
                for k in range(TILES):
                    ot = io_pool.tile([P, L * 2], f32, tag="ot")
                    if variant == "vy":
                        nc.sync.dma_start(ot[:], out_t[k])
                        lt = io_pool.tile([P, L], i32, tag="lt")
                        nc.sync.dma_start(lt[:], lab_t[k])
                    elif variant == "vz2":
                        ot_e = nc.sync if k % 2 == 0 else nc.scalar
                        ot_e.dma_start(ot[:], out_t[k])
                        lab_b = z_pool.tile([P, L], f16, tag="labb")
                        nc.gpsimd.dma_start(lab_b[:], lab_t[k])
                    else:
                        ot_e = nc.sync if k % 2 == 0 else nc.scalar
                        lab_e = nc.scalar if k % 2 == 0 else nc.sync
                        ot_e.dma_start(ot[:], out_t[k])
                        lt = io_pool.tile([P, L], i32, tag="lt")
                        lab_e.dma_start(lt[:], lab_t[k])
            nc.sync.dma_start(res_d[:], loss_t[:])
            done = True

        for _r in (range(repeat) if not done else ()):
            acc_S = acc_pool.tile([P, TILES], f32, tag="accS")
            acc_A = acc_pool.tile([P, TILES], f32, tag="accA")
            pend = []
            for k in range(TILES):
                ot = io_pool.tile([P, L * 2], f32, tag="ot")
                if ot_split == "sync":
                    nc.sync.dma_start(ot[:], out_t[k])
                elif ot_split == "alt":
                    eng = nc.sync if k % 2 == 0 else nc.scalar
                    eng.dma_start(ot[:], out_t[k])
                else:  # col: half the columns per HWDGE queue, every tile
                    nc.sync.dma_start(ot[:, 0:L], out_t[k][:, 0:L])
                    nc.scalar.dma_start(ot[:, L:L * 2], out_t[k][:, L:L * 2])

                if lab_path == "swdge":
                    # SWDGE cast DMA: reads i32 from HBM, writes f16 SBUF.
                    lab_b = z_pool.tile([P, L], f16, tag="labb")
                    nc.gpsimd.dma_start(lab_b[:], lab_t[k])
                else:
                    lab_eng = nc.sync if (
                        lab_path == "sync" or k % 2 == 1) else nc.scalar
                    lt = io_pool.tile([P, L], i32, tag="lt")
                    lab_eng.dma_start(lt[:], lab_t[k])
                    # ACT: int32 -> f16 cast on-engine.
                    lab_b = z_pool.tile([P, L], f16, tag="labb")
                    nc.scalar.activation(lab_b[:], lt[:], Act.Copy)

                x3 = ot[:].rearrange("p (l c) -> p l c", c=2)
                t0 = x3[:, :, 0]
                t1 = x3[:, :, 1]

                if variant in ("vp", "vq", "vr", "vs", "vt", "vu"):
                    # ACT: t1 -> contiguous f16 (makes the STT 2x).
                    t1h = m_pool.tile([P, L], f16, tag="t1h")
                    nc.scalar.activation(t1h[:], t1, Act.Copy)
                    if variant == "vq":
                        # vq: d on Pool from the raw strided f32.
                        d = d_pool.tile([P, L], f16)
                        nc.gpsimd.tensor_tensor(d[:], t0, t1, Op.subtract)
                    else:
                        # ACT: t0 -> f16 too; d on DVE (TT f16 2x).
                        t0h = m_pool.tile([P, L], f16, tag="t0h")
                        nc.scalar.activation(t0h[:], t0, Act.Copy)
                        if variant == "vs":
                            # in-place: d overwrites t0h (saves a pool)
                            d = t0h
                            nc.vector.tensor_tensor(
                                d[:], t0h[:], t1h[:], Op.subtract)
                        else:
                            d = d_pool.tile([P, L], f16)
                            nc.vector.tensor_tensor(
                                d[:], t0h[:], t1h[:], Op.subtract)

                    if variant in ("vr", "vs", "vt", "vu"):
                        M = M_pers[k % 2]
                    else:
                        M = w_pool.tile([P, L + 1], f16, tag="M")
                        nc.vector.memset(M[:, L:L + 1], -1.0)
                    nc.vector.tensor_tensor_scan(
                        M[:, 0:L][:, ::-1], d[:, ::-1], d[:, ::-1], -1.0,
                        Op.max, Op.max,
                    )
                    thr = acc_pool.tile([P, 1], f32, tag="thr")
                    nc.vector.tensor_scalar(
                        thr[:], M[:, 0:1], 0.0, -BIG, Op.is_lt, Op.mult
                    )
                    # DVE: w = (M[j+1] >= thr) * t1h -- all f16, 2x.
                    w = w_pool.tile([P, L], f16)
                    nc.vector.scalar_tensor_tensor(
                        w[:], M[:, 1:L + 1], thr[:], t1h[:],
                        Op.is_ge, Op.mult,
                        accum_out=acc_S[:, k:k + 1],
                    )
                    if variant == "vs":
                        # in-place: z overwrites lab_b (saves a pool)
                        z = lab_b
                        nc.vector.tensor_tensor(z[:], lab_b[:], pre_b[:],
                                                Op.mult)
                    else:
                        z = z_pool.tile([P, L], f16, tag="z")
                        nc.vector.tensor_tensor(z[:], lab_b[:], pre_b[:],
                                                Op.mult)
                    if variant == "vu":
                        # fused wz + A-accum: dummy broadcast out (stride
                        # 0), accum_out carries the real result.
                        dum = acc_pool.tile([P, 1], f16, tag="dumA")
                        nc.vector.tensor_tensor_reduce(
                            dum.broadcast_to((P, L)), w[:], z[:],
                            1.0, 0.0, Op.mult, Op.add,
                            accum_out=acc_A[:, k:k + 1],
                        )
                        continue
                    nc.vector.tensor_tensor(w[:], w[:], z[:], Op.mult)
                    if variant == "vt":
                        # defer sink(k) until after tile k+1's ACT casts
                        pend.append((w, k))
                        if len(pend) > 1:
                            w_p, k_p = pend.pop(0)
                            sink = w_pool.tile([P, L], f16, tag="sink")
                            nc.scalar.activation(
                                sink[:], w_p[:], Act.Copy,
                                accum_out=acc_A[:, k_p:k_p + 1],
                            )
                        continue
                    sink = w_pool.tile([P, L], f16, tag="sink")
                    nc.scalar.activation(
                        sink[:], w[:], Act.Copy,
                        accum_out=acc_A[:, k:k + 1],
                    )
                    continue

                if variant == "vn":
                    # scan-free mask: d = t0-t1 (Pool); y = (d>=0)*(j+1)
                    # in ONE DVE STT; m = max_j y = 1 + (last j with d>=0),
                    # or 0 if none; mask[j] = (j+1 < m'), m' = m (or L+1
                    # when m==0).  z = lab*preD on Pool (Q7 mult).
                    d = d_pool.tile([P, L], f16)
                    nc.gpsimd.tensor_tensor(d[:], t0, t1, Op.subtract)
                    # z on DVE (f16 2x) — the Q7's f16 multiply is slow
                    z = z_pool.tile([P, L], f16, tag="z")
                    nc.vector.tensor_tensor(z[:], lab_b[:], pre_b[:],
                                            Op.mult)
                    y = m_pool.tile([P, L], f32, tag="y")
                    nc.vector.scalar_tensor_tensor(
                        y[:], d[:], 0.0, iota1[:],
                        Op.is_ge, Op.mult,
                    )
                    mq = acc_pool.tile([P, 1], f32, tag="mq")
                    nc.vector.reduce_max(mq[:], y[:],
                                         axis=mybir.AxisListType.X)
                    # m' = m + (m==0)*(L+1), two tiny [P,1] ops
                    eq2 = acc_pool.tile([P, 1], f32, tag="eq2")
                    nc.vector.tensor_scalar(
                        eq2[:], mq[:], 0.0, float(L + 1), Op.is_equal,
                        Op.mult)
                    nc.vector.tensor_tensor(mq[:], mq[:], eq2[:], Op.add)
                    # w = (iota1 < m') * t1, S_k = sum(w)
                    w = w_pool.tile([P, L], f16)
                    nc.vector.scalar_tensor_tensor(
                        w[:], iota1[:], mq[:], t1,
                        Op.is_lt, Op.mult,
                        accum_out=acc_S[:, k:k + 1],
                    )
                    # w *= z (TT f16, 2x, in-place)
                    nc.vector.tensor_tensor(w[:], w[:], z[:], Op.mult)
                    # ACT: sink copy with accumulator -> A_k
                    sink = w_pool.tile([P, L], f16, tag="sink")
                    nc.scalar.activation(
                        sink[:], w[:], Act.Copy,
                        accum_out=acc_A[:, k:k + 1],
                    )
                    continue

                # d = t0 - t1 (f16 out).  Pool's Q7 software TT is slow on
                # strided f32 reads, so variants move this to ACT.
                d = d_pool.tile([P, L], f16)
                if sub_eng == "pool":
                    nc.gpsimd.tensor_tensor(d[:], t0, t1, Op.subtract)
                elif sub_eng == "dve":
                    nc.vector.tensor_tensor(d[:], t0, t1, Op.subtract)
                elif sub_eng == "psplit":
                    # low 3/8 on Pool (its Q7 sub is ~4x slower per elem),
                    # high 5/8 on DVE, so both finish together.
                    h = 3 * L // 8
                    nc.gpsimd.tensor_tensor(
                        d[:, 0:h], t0[:, 0:h], t1[:, 0:h], Op.subtract)
                    nc.vector.tensor_tensor(
                        d[:, h:L], t0[:, h:L], t1[:, h:L], Op.subtract)
                elif sub_eng == "act":
                    nc.scalar.tensor_tensor(d[:], t0, t1, Op.subtract)
                else:  # split: low half on Pool, high half on ACT
                    h = L // 2
                    nc.gpsimd.tensor_tensor(
                        d[:, 0:h], t0[:, 0:h], t1[:, 0:h], Op.subtract)
                    nc.scalar.tensor_tensor(
                        d[:, h:L], t0[:, h:L], t1[:, h:L], Op.subtract)

                # DVE scan: M[j] = max(d[j:], -1), M[L] = -1 pad (f16).
                # Ordered before z so ACT's sink(k-1) hides under the
                # scan+STT window instead of stalling DVE's queue head.
                M = m_pool.tile([P, L + 1], f16)
                nc.vector.memset(M[:, L:L + 1], -1.0)
                nc.vector.tensor_tensor_scan(
                    M[:, 0:L][:, ::-1], d[:, ::-1], d[:, ::-1], -1.0,
                    Op.max, Op.max,
                )

                # tiny (DVE): thr = 0 if M[0] >= 0 else -BIG, one fused TS:
                # (M0 < 0) * -BIG
                thr = acc_pool.tile([P, 1], f32, tag="thr")
                nc.vector.tensor_scalar(
                    thr[:], M[:, 0:1], 0.0, -BIG, Op.is_lt, Op.mult
                )

                # DVE: w = (M[j+1] >= thr) * t1 (f16 out), S_k = sum(w).
                w = w_pool.tile([P, L], f16)
                nc.vector.scalar_tensor_tensor(
                    w[:], M[:, 1:L + 1], thr[:], t1,
                    Op.is_ge, Op.mult,
                    accum_out=acc_S[:, k:k + 1],
                )

                # z = lab_b * preD (TT f16): on Pool for vo (frees DVE),
                # else DVE (2x), late on purpose.
                z = z_pool.tile([P, L], f16, tag="z")
                z_eng = nc.gpsimd if variant == "vo" else nc.vector
                z_eng.tensor_tensor(z[:], lab_b[:], pre_b[:], Op.mult)

                if use_ttr:
                    # DVE: w = w*z fused with A_k = sum(w*z); no ACT sink.
                    nc.vector.tensor_tensor_reduce(
                        w[:], w[:], z[:], 1.0, 0.0, Op.mult, Op.add,
                        accum_out=acc_A[:, k:k + 1],
                    )
                else:
                    # DVE: w *= z (TT f16, 2x, in-place).
                    nc.vector.tensor_tensor(w[:], w[:], z[:], Op.mult)

                    # ACT: sink copy with accumulator -> A_k = sum(w*z).
                    sink = w_pool.tile([P, L], f16, tag="sink")
                    nc.scalar.activation(
                        sink[:], w[:], Act.Copy,
                        accum_out=acc_A[:, k:k + 1],
                    )

            for w_p, k_p in pend:
                sink = w_pool.tile([P, L], f16, tag="sink")
                nc.scalar.activation(
                    sink[:], w_p[:], Act.Copy,
                    accum_out=acc_A[:, k_p:k_p + 1],
                )

            if k == TILES - 1:
                # tail: loss_i = C*sum_k S_k + sum_k A_k
                t4 = acc_pool.tile([P, TILES], f32, tag="t4")
                nc.vector.tensor_scalar(t4[:], acc_S[:], C_CONST, None, Op.mult)
                nc.vector.tensor_tensor(t4[:], t4[:], acc_A[:], Op.add)
                loss_t = acc_pool.tile([P, 1], f32, tag="loss")
                nc.vector.reduce_sum(loss_t[:], t4[:], axis=mybir.AxisListType.X)

        if not done:
            nc.sync.dma_start(res_d[:], loss_t[:])

    nc.compile()
    return nc


def _pre_tile() -> np.ndarray:
    j = np.arange(L, dtype=np.float64)
    pre2 = (-3.6 / np.log2(j + 2.0) - C_CONST).astype(np.float32)
    iota1 = (j + 1.0).astype(np.float32)
    plane = np.concatenate([pre2, iota1])
    return np.ascontiguousarray(np.tile(plane[None, :], (P, 1)))


def _get_nc(repeat: int = 1, variant: str = VARIANT):
    key = (repeat, variant)
    if key not in _CACHE:
        _CACHE[key] = _build_nc(repeat=repeat, variant=variant)
    return _CACHE[key]


def make_in_maps(output: np.ndarray, labels: np.ndarray):
    pre = _pre_tile()
    in_maps = []
    for c in range(N_CORES):
        sl = slice(c * ROWS_PER_CORE, (c + 1) * ROWS_PER_CORE)
        in_maps.append({
            "out": np.ascontiguousarray(output[sl]).reshape(ROWS_PER_CORE, L * 2),
            "lab": np.ascontiguousarray(labels[sl]),
            "pre": pre,
        })
    return in_maps


def kernel(output: np.ndarray, labels: np.ndarray) -> np.ndarray:
    from concourse.bass_utils import run_bass_kernel_spmd

    nc = _get_nc(repeat=1)
    in_maps = make_in_maps(output, labels)
    r = run_bass_kernel_spmd(nc, in_maps, core_ids=list(range(N_CORES)))
    total = 0.0
    for res in r.results:
        total += float(res["res"].astype(np.float64).sum())
    return np.float32(total / B)


if __name__ == "__main__":
    # quick standalone run (full inputs, random)
    rng = np.random.default_rng(0)
    out = rng.standard_normal((B, L, 2)).astype(np.float32)
    lab = rng.integers(0, 2, size=(B, L)).astype(np.int32)
    print("loss:", kernel(out, lab))


# revision 78
# speedup vs baseline: 1.0761x; 1.0034x over previous
"""BiCutLoss Trainium2 kernel (nn_BiCutLoss_52312701665760).

Reference computation (per batch row i of output[B, L, 2], labels[B, L]):
  temp = argmax(output, -1)            # 1 iff out1 > out0
  cut  = L if all(temp == 1) else (index of last 0 in temp)
  mask = arange(L) < cut
  r1   = where(labels == 1, -3.6/log2(j+2), 0.065)
  loss = sum(out1 * mask * r1) / B

Kernel formulation (exactly equivalent):
  d[j] = out0[j] - out1[j]                       # temp[j]==0  <=>  d[j] >= 0
  M[j] = max(d[j:], -1)  (reverse cummax; M[L] = -1 pad)
  thr  = 0 if M[0] >= 0 else -BIG                # all-ones row => mask all 1
  mask[j] = (M[j+1] >= thr)
  r1   = C + lab*preD   with C = 0.065, preD[j] = -3.6/log2(j+2) - C
  S_i  = sum_j mask*t1          A_i = sum_j mask*t1*lab*preD
  loss_i = C*S_i + A_i

Sharding: pure data parallel - B=4096 rows split as 512 rows x 8 cores; each
core computes per-row partials [128,1] (4 row-tiles of 128 partitions), host
sums and divides by B.

Per-core HBM traffic: out 16.8 MB + lab 8.4 MB = 25.2 MB; measured DMA-only
floor (vz/vz2 probes) ~71.5-73 us/iter. Winning variant "vr" (~85.3 us/iter
in a clean window, measured via interleaved repeat-delta at R=129 vs 33):
  - ot [128, 8192] f32 DMA alternates SP/ACT HWDGE queues per tile
  - lab via gpsimd SWDGE cast-DMA i32->f16 (no ACT cast op; cast-DMA is
    not slower than raw, vz2 probe)
  - ACT casts t0/t1 (strided f32 -> contiguous f16 t0h/t1h); d = t0h-t1h
    on DVE as a cheap f16 2x TT
  - DVE (all f16, ~13us/tile): reverse scan max (~8.2us, irreducible),
    thr (tiny), w = (M[1:] >= thr)*t1h (STT 2x, accum -> S_k),
    z = lab_b*preD (TT 2x), w *= z (TT 2x)
  - M pad column preset once in two persistent buffers (no per-tile
    memset); ACT: sink Copy(w) accum -> A_k

Measured dead ends (same-process interleaved races): all-DMA-on-one-queue
(v0, +5us; each engine's HWDGE already spreads over 8 HW queues), SWDGE
vs HWDGE lab (±1us), io bufs=3 (±0), column-split DMA (±0), sub all-DVE
(+3us) or all-Pool (+2..6us: Pool->DVE handoff latency), scan-free
iota-max formulation (vn, +11us), z/sub on Pool in f16 (vo, +29us: the
Q7's f16 multiply is far slower than its f32 subtract), in-place d/z +
io bufs=3 (vs, +36us: in-place WAR serialization), deferring the ACT
sink one tile (vt, ±0), tensor_tensor_reduce in any form including the
qr.py dummy-broadcast pattern (crashes the device mesh).
"""

import os
from contextlib import ExitStack

import numpy as np

B, L = 4096, 4096
N_CORES = 8
ROWS_PER_CORE = B // N_CORES          # 512
P = 128                               # partitions per tile
TILES = ROWS_PER_CORE // P            # 4
C_CONST = 0.65 * 0.1                  # 0.065
BIG = 1e30

VARIANT = "vr"                        # kernel() uses this one

_CACHE = {}


def _build_nc(repeat: int = 1, variant: str = VARIANT):
    import concourse.mybir as mybir
    import concourse.tile as tile
    from concourse import bacc

    f32 = mybir.dt.float32
    f16 = mybir.dt.float16
    i32 = mybir.dt.int32
    Op = mybir.AluOpType
    Act = mybir.ActivationFunctionType

    # variant knobs
    #   ot_split: how ot's 4.19 MB/tile is routed over the two HWDGE queues
    #   lab_path: lab DMA queue + where the i32->f16 cast happens
    #   io_bufs:  io pool depth (DMA pipelining)
    #   use_ttr:  fuse w*z and the A-reduction into one DVE TTR (no ACT sink)
    knobs = {
        #         ot_split  lab_path  io_bufs  use_ttr  sub_eng
        "v0": ("sync",    "sync",   2, False, "pool"),
        "va": ("alt",     "alt",    2, False, "pool"),
        "vb": ("alt",     "swdge",  2, False, "pool"),
        "vd": ("col",     "swdge",  2, False, "pool"),
        "ve": ("alt",     "swdge",  3, False, "pool"),
        "vf": ("alt",     "swdge",  2, True,  "pool"),
        "vg": ("alt",     "swdge",  2, False, "act"),
        "vh": ("alt",     "swdge",  2, False, "split"),
        "vi": ("alt",     "alt",    2, False, "act"),
        "vj": ("alt",     "swdge",  2, False, "psplit"),
        "vk": ("alt",     "swdge",  2, False, "dve"),
        "vl": ("alt",     "alt",    2, False, "dve"),
        # vm = vj + lean: persistent M pad (no per-tile memset) and raw
        # acc_S/acc_A output with the C*S+A tail computed on host.
        "vm": ("alt",     "swdge",  2, False, "psplit"),
        # vn = scan-free: cut via m = max((j+1)*(d>=0)) fused in one STT;
        # d and z on Pool. Kills the 8.2us scan + memset + thr.
        "vn": ("alt",     "swdge",  2, False, "pool"),
        # vo = scan path, but sub AND z both on Pool: DVE keeps only
        # scan + STT + wz (+ tiny), ~14.6us/tile.
        "vo": ("alt",     "swdge",  2, False, "pool"),
        # vp/vq = all-f16 DVE: ACT casts t1 (and t0 for vp) to contiguous
        # f16 so the STT gets 2x; sub on DVE-f16 (vp) or Pool-f32 (vq).
        "vp": ("alt",     "swdge",  2, False, "dve"),
        "vq": ("alt",     "swdge",  2, False, "pool"),
        # vr = vp + persistent M buffers (pad memset once, not per tile).
        "vr": ("alt",     "swdge",  2, False, "dve"),
        # vs = vr + in-place d (into t0h) and z (into lab_b) + io bufs=3.
        "vs": ("alt",     "swdge",  3, False, "dve"),
        # vt = vr + sink deferred one tile, so ACT's casts for tile k+1
        # aren't queued behind sink(k) (which waits on wz(k)).
        "vt": ("alt",     "swdge",  2, False, "dve"),
        # vu = vr + wz fused with the A-accum in one dummy-broadcast TTR
        # on DVE (qr.py pattern); ACT does only the two casts.
        "vu": ("alt",     "swdge",  2, False, "dve"),
        # vw = vr + column-split ot DMA (both HWDGE queues fill each tile
        # simultaneously, halving the arrival latency that gates casts).
        "vw": ("col",     "swdge",  2, False, "dve"),
        # vx = vw + half-casts: each ACT cast covers half the columns and
        # starts as soon as its DMA half lands.
        "vx": ("col",     "swdge",  2, False, "dve"),
        # wa = vr + DVE queue reorder: z (depends only on the early SWDGE
        # lab DMA) is emitted before d, absorbing the ACT-cast latency.
        "wa": ("alt",     "swdge",  2, False, "dve"),
    }
    probes = ("vz", "vy", "vz2", "pscan", "psub", "pstt", "ptt", "pred",
              "ppool", "pact")
    if variant in probes:
        ot_split, lab_path, io_bufs, use_ttr, sub_eng = (
            "alt", "swdge", 2, False, "pool")
    else:
        ot_split, lab_path, io_bufs, use_ttr, sub_eng = knobs[variant]

    # Bacc (not raw Bass): its compile() runs generate_event_semaphores,
    # which splits multi-sem waits into standalone EventSemaphore
    # instructions (HW allows at most 1 wait per compute instruction).
    nc = bacc.Bacc("TRN2", target_bir_lowering=False, debug=False)

    out_d = nc.dram_tensor("out", [ROWS_PER_CORE, L * 2], f32, kind="ExternalInput")
    lab_d = nc.dram_tensor("lab", [ROWS_PER_CORE, L], i32, kind="ExternalInput")
    # pre holds two [P, L] constant planes: [:, 0:L] = preD, [:, L:2L] = j+1
    pre_d = nc.dram_tensor("pre", [P, L * 2], f32, kind="ExternalInput")
    res_d = nc.dram_tensor("res", [P, 1], f32, kind="ExternalOutput")

    out_t = out_d[:].rearrange("(n p) m -> n p m", p=P)   # [4, 128, 8192]
    lab_t = lab_d[:].rearrange("(n p) m -> n p m", p=P)   # [4, 128, 4096]

    with tile.TileContext(nc) as tc, ExitStack() as ctx:
        io_pool = ctx.enter_context(tc.tile_pool(name="io", bufs=io_bufs))
        pre_pool = ctx.enter_context(tc.tile_pool(name="pre", bufs=1))
        d_pool = ctx.enter_context(tc.tile_pool(name="d", bufs=2))
        m_pool = ctx.enter_context(tc.tile_pool(name="m", bufs=2))
        w_pool = ctx.enter_context(tc.tile_pool(name="w", bufs=2))
        z_pool = ctx.enter_context(tc.tile_pool(name="z", bufs=2))
        acc_pool = ctx.enter_context(tc.tile_pool(name="acc", bufs=2))

        done = False
        # preamble (outside the repeat loop; excluded from per-iter time):
        # preD as f16 via SWDGE cast DMA, one-time.
        pre_b = pre_pool.tile([P, L], f16)
        if variant not in ("pscan", "psub", "pstt", "ptt", "pred", "ppool",
                           "pact"):
            nc.gpsimd.dma_start(pre_b[:], pre_d[:][:, 0:L])
        if variant == "vn":
            iota1 = pre_pool.tile([P, L], f32, tag="iota1")
            nc.sync.dma_start(iota1[:], pre_d[:][:, L:L * 2])
        if variant in ("vr", "vs", "vt", "vu", "vw", "vx", "wa"):
            M_a = pre_pool.tile([P, L + 1], f16, tag="Mpa")
            M_b = pre_pool.tile([P, L + 1], f16, tag="Mpb")
            M_pers = [M_a, M_b]
            nc.vector.memset(M_a[:, L:L + 1], -1.0)
            nc.vector.memset(M_b[:, L:L + 1], -1.0)

        if variant in ("pscan", "psub", "pstt", "ptt", "pred", "ppool",
                       "pact"):
            pre_f = pre_pool.tile([P, L], f32, tag="pref")
            nc.sync.dma_start(pre_f[:], pre_d[:][:, 0:L])
            nc.scalar.activation(pre_b[:], pre_f[:], Act.Copy)
            # Engine-primitive probes: one op per repeat-iteration on
            # resident SBUF data (loaded once in the preamble). The
            # repeat-R-vs-repeat-r delta gives the op's true per-call cost.
            # Result is garbage by design.
            # All scratch allocated ONCE (bufs=1 pool); the loop rewrites
            # the same tiles from a single engine, so iterations serialize
            # in program order with no semaphores (pure op-rate measure).
            ot = pre_pool.tile([P, L * 2], f32, tag="pot")
            nc.sync.dma_start(ot[:], out_t[0])
            # single-HWDGE-queue preamble: the SP drain can only carry 4
            # sem waits, so probes avoid SWDGE/ACT queues entirely.
            lt0 = pre_pool.tile([P, L], i32, tag="plt0")
            nc.sync.dma_start(lt0[:], lab_t[0])
            lab_b = pre_pool.tile([P, L], f16, tag="plabb")
            nc.scalar.activation(lab_b[:], lt0[:], Act.Copy)
            x3 = ot[:].rearrange("p (l c) -> p l c", c=2)
            t0, t1 = x3[:, :, 0], x3[:, :, 1]
            d0 = pre_pool.tile([P, L], f16, tag="pd0")
            nc.vector.tensor_tensor(d0[:], t0, t1, Op.subtract)
            thr0 = pre_pool.tile([P, 1], f32, tag="pthr0")
            nc.vector.memset(thr0[:], 0.0)
            M = pre_pool.tile([P, L + 1], f16, tag="pM")
            nc.vector.memset(M[:, L:L + 1], -1.0)
            d = pre_pool.tile([P, L], f16, tag="pd")
            acc = pre_pool.tile([P, 1], f32, tag="pacc")
            w = pre_pool.tile([P, L], f16, tag="pw")
            z = pre_pool.tile([P, L], f16, tag="pz")
            mx = pre_pool.tile([P, 1], f32, tag="pmx")
            mxh = pre_pool.tile([P, 1], f16, tag="pmxh")
            loss_t = pre_pool.tile([P, 1], f32, tag="ploss")
            nc.vector.memset(loss_t[:], 0.0)
            for _r in range(repeat):
                if variant == "pscan":
                    nc.vector.tensor_tensor_scan(
                        M[:, 0:L][:, ::-1], d0[:, ::-1], d0[:, ::-1], -1.0,
                        Op.max, Op.max,
                    )
                    tgt = M
                elif variant == "psub":
                    nc.gpsimd.tensor_tensor(d[:], t0, t1, Op.subtract)
                    tgt = d
                elif variant == "pstt":
                    nc.vector.scalar_tensor_tensor(
                        w[:], d0[:], thr0[:], t1,
                        Op.is_ge, Op.mult,
                        accum_out=acc[:],
                    )
                    tgt = w
                elif variant == "ptt":
                    nc.vector.tensor_tensor(z[:], lab_b[:], pre_b[:], Op.mult)
                    tgt = z
                elif variant == "pred":
                    nc.vector.reduce_max(mx[:], d0[:], axis=mybir.AxisListType.X)
                    tgt = mx
                elif variant == "ppool":
                    # InstPool max on DVE (vs pred's tensor_reduce max)
                    nc.vector.pool_max(mxh[:], d0[:])
                    tgt = mxh
                else:  # pact: ACT sink copy with accum
                    nc.scalar.activation(
                        w[:], d0[:], Act.Copy,
                        accum_out=acc[:],
                    )
                    tgt = w
                # cheap consumer: keeps every write consumed so the final
                # Drain doesn't accumulate unbounded sem waits.
                nc.vector.tensor_tensor(
                    loss_t[:], loss_t[:], tgt[:, 0:1], Op.add)
            nc.sync.dma_start(res_d[:], loss_t[:])
            done = True

        if variant in ("vz", "vy", "vz2"):
            # DMA-only probe: measures the pure HBM streaming floor.
            # vz: balanced SP/ACT HWDGE queues (12.6 MB each); vy: all on
            # SP; vz2: the vj layout (alt ot + SWDGE cast-DMA lab).
            # Result is garbage (zeros) by design.
            loss_t = acc_pool.tile([P, 1], f32, tag="loss")
            nc.vector.memset(loss_t[:], 0.0)
            for _r in range(repeat):
                for k in range(TILES):
                    ot = io_pool.tile([P, L * 2], f32, tag="ot")
                    if variant == "vy":
                        nc.sync.dma_start(ot[:], out_t[k])
                        lt = io_pool.tile([P, L], i32, tag="lt")
                        nc.sync.dma_start(lt[:], lab_t[k])
                    elif variant == "vz2":
                        ot_e = nc.sync if k % 2 == 0 else nc.scalar
                        ot_e.dma_start(ot[:], out_t[k])
                        lab_b = z_pool.tile([P, L], f16, tag="labb")
                        nc.gpsimd.dma_start(lab_b[:], lab_t[k])
                    else:
                        ot_e = nc.sync if k % 2 == 0 else nc.scalar
                        lab_e = nc.scalar if k % 2 == 0 else nc.sync
                        ot_e.dma_start(ot[:], out_t[k])
                        lt = io_pool.tile([P, L], i32, tag="lt")
                        lab_e.dma_start(lt[:], lab_t[k])
            nc.sync.dma_start(res_d[:], loss_t[:])
            done = True

        for _r in (range(repeat) if not done else ()):
            acc_S = acc_pool.tile([P, TILES], f32, tag="accS")
            acc_A = acc_pool.tile([P, TILES], f32, tag="accA")
            pend = []
            for k in range(TILES):
                ot = io_pool.tile([P, L * 2], f32, tag="ot")
                if ot_split == "sync":
                    nc.sync.dma_start(ot[:], out_t[k])
                elif ot_split == "alt":
                    eng = nc.sync if k % 2 == 0 else nc.scalar
                    eng.dma_start(ot[:], out_t[k])
                else:  # col: half the columns per HWDGE queue, every tile
                    nc.sync.dma_start(ot[:, 0:L], out_t[k][:, 0:L])
                    nc.scalar.dma_start(ot[:, L:L * 2], out_t[k][:, L:L * 2])

                if lab_path == "swdge":
                    # SWDGE cast DMA: reads i32 from HBM, writes f16 SBUF.
                    lab_b = z_pool.tile([P, L], f16, tag="labb")
                    nc.gpsimd.dma_start(lab_b[:], lab_t[k])
                else:
                    lab_eng = nc.sync if (
                        lab_path == "sync" or k % 2 == 1) else nc.scalar
                    lt = io_pool.tile([P, L], i32, tag="lt")
                    lab_eng.dma_start(lt[:], lab_t[k])
                    # ACT: int32 -> f16 cast on-engine.
                    lab_b = z_pool.tile([P, L], f16, tag="labb")
                    nc.scalar.activation(lab_b[:], lt[:], Act.Copy)

                x3 = ot[:].rearrange("p (l c) -> p l c", c=2)
                t0 = x3[:, :, 0]
                t1 = x3[:, :, 1]

                if variant in ("vp", "vq", "vr", "vs", "vt", "vu", "vw",
                               "vx", "wa"):
                    if variant == "wa":
                        # z first in DVE's queue: it depends only on the
                        # early SWDGE lab DMA, so DVE does useful work
                        # while ACT is still casting t0/t1.
                        z = z_pool.tile([P, L], f16, tag="z")
                        nc.vector.tensor_tensor(z[:], lab_b[:], pre_b[:],
                                                Op.mult)
                    # ACT: t1 -> contiguous f16 (makes the STT 2x).
                    t1h = m_pool.tile([P, L], f16, tag="t1h")
                    if variant == "vx":
                        h = L // 2
                        nc.scalar.activation(
                            t1h[:, 0:h], x3[:, 0:h, 1], Act.Copy)
                        nc.scalar.activation(
                            t1h[:, h:L], x3[:, h:L, 1], Act.Copy)
                    else:
                        nc.scalar.activation(t1h[:], t1, Act.Copy)
                    if variant == "vq":
                        # vq: d on Pool from the raw strided f32.
                        d = d_pool.tile([P, L], f16)
                        nc.gpsimd.tensor_tensor(d[:], t0, t1, Op.subtract)
                    else:
                        # ACT: t0 -> f16 too; d on DVE (TT f16 2x).
                        t0h = m_pool.tile([P, L], f16, tag="t0h")
                        if variant == "vx":
                            h = L // 2
                            nc.scalar.activation(
                                t0h[:, 0:h], x3[:, 0:h, 0], Act.Copy)
                            nc.scalar.activation(
                                t0h[:, h:L], x3[:, h:L, 0], Act.Copy)
                        else:
                            nc.scalar.activation(t0h[:], t0, Act.Copy)
                        if variant == "vs":
                            # in-place: d overwrites t0h (saves a pool)
                            d = t0h
                            nc.vector.tensor_tensor(
                                d[:], t0h[:], t1h[:], Op.subtract)
                        else:
                            d = d_pool.tile([P, L], f16)
                            nc.vector.tensor_tensor(
                                d[:], t0h[:], t1h[:], Op.subtract)

                    if variant in ("vr", "vs", "vt", "vu", "vw", "vx",
                                   "wa"):
                        M = M_pers[k % 2]
                    else:
                        M = w_pool.tile([P, L + 1], f16, tag="M")
                        nc.vector.memset(M[:, L:L + 1], -1.0)
                    nc.vector.tensor_tensor_scan(
                        M[:, 0:L][:, ::-1], d[:, ::-1], d[:, ::-1], -1.0,
                        Op.max, Op.max,
                    )
                    thr = acc_pool.tile([P, 1], f32, tag="thr")
                    nc.vector.tensor_scalar(
                        thr[:], M[:, 0:1], 0.0, -BIG, Op.is_lt, Op.mult
                    )
                    # DVE: w = (M[j+1] >= thr) * t1h -- all f16, 2x.
                    w = w_pool.tile([P, L], f16)
                    nc.vector.scalar_tensor_tensor(
                        w[:], M[:, 1:L + 1], thr[:], t1h[:],
                        Op.is_ge, Op.mult,
                        accum_out=acc_S[:, k:k + 1],
                    )
                    if variant == "vs":
                        # in-place: z overwrites lab_b (saves a pool)
                        z = lab_b
                        nc.vector.tensor_tensor(z[:], lab_b[:], pre_b[:],
                                                Op.mult)
                    elif variant != "wa":  # wa emitted z before the casts
                        z = z_pool.tile([P, L], f16, tag="z")
                        nc.vector.tensor_tensor(z[:], lab_b[:], pre_b[:],
                                                Op.mult)
                    if variant == "vu":
                        # fused wz + A-accum: dummy broadcast out (stride
                        # 0), accum_out carries the real result.
                        dum = acc_pool.tile([P, 1], f16, tag="dumA")
                        nc.vector.tensor_tensor_reduce(
                            dum.broadcast_to((P, L)), w[:], z[:],
                            1.0, 0.0, Op.mult, Op.add,
                            accum_out=acc_A[:, k:k + 1],
                        )
                        continue
                    nc.vector.tensor_tensor(w[:], w[:], z[:], Op.mult)
                    if variant == "vt":
                        # defer sink(k) until after tile k+1's ACT casts
                        pend.append((w, k))
                        if len(pend) > 1:
                            w_p, k_p = pend.pop(0)
                            sink = w_pool.tile([P, L], f16, tag="sink")
                            nc.scalar.activation(
                                sink[:], w_p[:], Act.Copy,
                                accum_out=acc_A[:, k_p:k_p + 1],
                            )
                        continue
                    sink = w_pool.tile([P, L], f16, tag="sink")
                    nc.scalar.activation(
                        sink[:], w[:], Act.Copy,
                        accum_out=acc_A[:, k:k + 1],
                    )
                    continue

                if variant == "vn":
                    # scan-free mask: d = t0-t1 (Pool); y = (d>=0)*(j+1)
                    # in ONE DVE STT; m = max_j y = 1 + (last j with d>=0),
                    # or 0 if none; mask[j] = (j+1 < m'), m' = m (or L+1
                    # when m==0).  z = lab*preD on Pool (Q7 mult).
                    d = d_pool.tile([P, L], f16)
                    nc.gpsimd.tensor_tensor(d[:], t0, t1, Op.subtract)
                    # z on DVE (f16 2x) — the Q7's f16 multiply is slow
                    z = z_pool.tile([P, L], f16, tag="z")
                    nc.vector.tensor_tensor(z[:], lab_b[:], pre_b[:],
                                            Op.mult)
                    y = m_pool.tile([P, L], f32, tag="y")
                    nc.vector.scalar_tensor_tensor(
                        y[:], d[:], 0.0, iota1[:],
                        Op.is_ge, Op.mult,
                    )
                    mq = acc_pool.tile([P, 1], f32, tag="mq")
                    nc.vector.reduce_max(mq[:], y[:],
                                         axis=mybir.AxisListType.X)
                    # m' = m + (m==0)*(L+1), two tiny [P,1] ops
                    eq2 = acc_pool.tile([P, 1], f32, tag="eq2")
                    nc.vector.tensor_scalar(
                        eq2[:], mq[:], 0.0, float(L + 1), Op.is_equal,
                        Op.mult)
                    nc.vector.tensor_tensor(mq[:], mq[:], eq2[:], Op.add)
                    # w = (iota1 < m') * t1, S_k = sum(w)
                    w = w_pool.tile([P, L], f16)
                    nc.vector.scalar_tensor_tensor(
                        w[:], iota1[:], mq[:], t1,
                        Op.is_lt, Op.mult,
                        accum_out=acc_S[:, k:k + 1],
                    )
                    # w *= z (TT f16, 2x, in-place)
                    nc.vector.tensor_tensor(w[:], w[:], z[:], Op.mult)
                    # ACT: sink copy with accumulator -> A_k
                    sink = w_pool.tile([P, L], f16, tag="sink")
                    nc.scalar.activation(
                        sink[:], w[:], Act.Copy,
                        accum_out=acc_A[:, k:k + 1],
                    )
                    continue

                # d = t0 - t1 (f16 out).  Pool's Q7 software TT is slow on
                # strided f32 reads, so variants move this to ACT.
                d = d_pool.tile([P, L], f16)
                if sub_eng == "pool":
                    nc.gpsimd.tensor_tensor(d[:], t0, t1, Op.subtract)
                elif sub_eng == "dve":
                    nc.vector.tensor_tensor(d[:], t0, t1, Op.subtract)
                elif sub_eng == "psplit":
                    # low 3/8 on Pool (its Q7 sub is ~4x slower per elem),
                    # high 5/8 on DVE, so both finish together.
                    h = 3 * L // 8
                    nc.gpsimd.tensor_tensor(
                        d[:, 0:h], t0[:, 0:h], t1[:, 0:h], Op.subtract)
                    nc.vector.tensor_tensor(
                        d[:, h:L], t0[:, h:L], t1[:, h:L], Op.subtract)
                elif sub_eng == "act":
                    nc.scalar.tensor_tensor(d[:], t0, t1, Op.subtract)
                else:  # split: low half on Pool, high half on ACT
                    h = L // 2
                    nc.gpsimd.tensor_tensor(
                        d[:, 0:h], t0[:, 0:h], t1[:, 0:h], Op.subtract)
                    nc.scalar.tensor_tensor(
                        d[:, h:L], t0[:, h:L], t1[:, h:L], Op.subtract)

                # DVE scan: M[j] = max(d[j:], -1), M[L] = -1 pad (f16).
                # Ordered before z so ACT's sink(k-1) hides under the
                # scan+STT window instead of stalling DVE's queue head.
                M = m_pool.tile([P, L + 1], f16)
                nc.vector.memset(M[:, L:L + 1], -1.0)
                nc.vector.tensor_tensor_scan(
                    M[:, 0:L][:, ::-1], d[:, ::-1], d[:, ::-1], -1.0,
                    Op.max, Op.max,
                )

                # tiny (DVE): thr = 0 if M[0] >= 0 else -BIG, one fused TS:
                # (M0 < 0) * -BIG
                thr = acc_pool.tile([P, 1], f32, tag="thr")
                nc.vector.tensor_scalar(
                    thr[:], M[:, 0:1], 0.0, -BIG, Op.is_lt, Op.mult
                )

                # DVE: w = (M[j+1] >= thr) * t1 (f16 out), S_k = sum(w).
                w = w_pool.tile([P, L], f16)
                nc.vector.scalar_tensor_tensor(
                    w[:], M[:, 1:L + 1], thr[:], t1,
                    Op.is_ge, Op.mult,
                    accum_out=acc_S[:, k:k + 1],
                )

                # z = lab_b * preD (TT f16): on Pool for vo (frees DVE),
                # else DVE (2x), late on purpose.
                z = z_pool.tile([P, L], f16, tag="z")
                z_eng = nc.gpsimd if variant == "vo" else nc.vector
                z_eng.tensor_tensor(z[:], lab_b[:], pre_b[:], Op.mult)

                if use_ttr:
                    # DVE: w = w*z fused with A_k = sum(w*z); no ACT sink.
                    nc.vector.tensor_tensor_reduce(
                        w[:], w[:], z[:], 1.0, 0.0, Op.mult, Op.add,
                        accum_out=acc_A[:, k:k + 1],
                    )
                else:
                    # DVE: w *= z (TT f16, 2x, in-place).
                    nc.vector.tensor_tensor(w[:], w[:], z[:], Op.mult)

                    # ACT: sink copy with accumulator -> A_k = sum(w*z).
                    sink = w_pool.tile([P, L], f16, tag="sink")
                    nc.scalar.activation(
                        sink[:], w[:], Act.Copy,
                        accum_out=acc_A[:, k:k + 1],
                    )

            for w_p, k_p in pend:
                sink = w_pool.tile([P, L], f16, tag="sink")
                nc.scalar.activation(
                    sink[:], w_p[:], Act.Copy,
                    accum_out=acc_A[:, k_p:k_p + 1],
                )

            if k == TILES - 1:
                # tail: loss_i = C*sum_k S_k + sum_k A_k
                t4 = acc_pool.tile([P, TILES], f32, tag="t4")
                nc.vector.tensor_scalar(t4[:], acc_S[:], C_CONST, None, Op.mult)
                nc.vector.tensor_tensor(t4[:], t4[:], acc_A[:], Op.add)
                loss_t = acc_pool.tile([P, 1], f32, tag="loss")
                nc.vector.reduce_sum(loss_t[:], t4[:], axis=mybir.AxisListType.X)

        if not done:
            nc.sync.dma_start(res_d[:], loss_t[:])

    nc.compile()
    return nc


def _pre_tile() -> np.ndarray:
    j = np.arange(L, dtype=np.float64)
    pre2 = (-3.6 / np.log2(j + 2.0) - C_CONST).astype(np.float32)
    iota1 = (j + 1.0).astype(np.float32)
    plane = np.concatenate([pre2, iota1])
    return np.ascontiguousarray(np.tile(plane[None, :], (P, 1)))


def _get_nc(repeat: int = 1, variant: str = VARIANT):
    key = (repeat, variant)
    if key not in _CACHE:
        _CACHE[key] = _build_nc(repeat=repeat, variant=variant)
    return _CACHE[key]


def make_in_maps(output: np.ndarray, labels: np.ndarray):
    pre = _pre_tile()
    in_maps = []
    for c in range(N_CORES):
        sl = slice(c * ROWS_PER_CORE, (c + 1) * ROWS_PER_CORE)
        in_maps.append({
            "out": np.ascontiguousarray(output[sl]).reshape(ROWS_PER_CORE, L * 2),
            "lab": np.ascontiguousarray(labels[sl]),
            "pre": pre,
        })
    return in_maps


def kernel(output: np.ndarray, labels: np.ndarray) -> np.ndarray:
    from concourse.bass_utils import run_bass_kernel_spmd

    nc = _get_nc(repeat=1)
    in_maps = make_in_maps(output, labels)
    r = run_bass_kernel_spmd(nc, in_maps, core_ids=list(range(N_CORES)))
    total = 0.0
    for res in r.results:
        total += float(res["res"].astype(np.float64).sum())
    return np.float32(total / B)


if __name__ == "__main__":
    # quick standalone run (full inputs, random)
    rng = np.random.default_rng(0)
    out = rng.standard_normal((B, L, 2)).astype(np.float32)
    lab = rng.integers(0, 2, size=(B, L)).astype(np.int32)
    print("loss:", kernel(out, lab))


# revision 98
# speedup vs baseline: 1.0767x; 1.0006x over previous
"""BiCutLoss Trainium2 kernel (nn_BiCutLoss_52312701665760).

Reference computation (per batch row i of output[B, L, 2], labels[B, L]):
  temp = argmax(output, -1)            # 1 iff out1 > out0
  cut  = L if all(temp == 1) else (index of last 0 in temp)
  mask = arange(L) < cut
  r1   = where(labels == 1, -3.6/log2(j+2), 0.065)
  loss = sum(out1 * mask * r1) / B

Kernel formulation (exactly equivalent):
  d[j] = out0[j] - out1[j]                       # temp[j]==0  <=>  d[j] >= 0
  M[j] = max(d[j:], -1)  (reverse cummax; M[L] = -1 pad)
  thr  = 0 if M[0] >= 0 else -BIG                # all-ones row => mask all 1
  mask[j] = (M[j+1] >= thr)
  r1   = C + lab*preD   with C = 0.065, preD[j] = -3.6/log2(j+2) - C
  S_i  = sum_j mask*t1          A_i = sum_j mask*t1*lab*preD
  loss_i = C*S_i + A_i

Sharding: pure data parallel - B=4096 rows split as 512 rows x 8 cores; each
core computes per-row partials [128,1] (4 row-tiles of 128 partitions), host
sums and divides by B.

Per-core HBM traffic: out 16.8 MB + lab 8.4 MB = 25.2 MB; measured DMA-only
floor (vz/vz2 probes) ~71.5-73 us/iter. Winning variant "vr" (~85.3 us/iter
in a clean window, measured via interleaved repeat-delta at R=129 vs 33):
  - ot [128, 8192] f32 DMA alternates SP/ACT HWDGE queues per tile
  - lab via gpsimd SWDGE cast-DMA i32->f16 (no ACT cast op; cast-DMA is
    not slower than raw, vz2 probe)
  - ACT casts t0/t1 (strided f32 -> contiguous f16 t0h/t1h); d = t0h-t1h
    on DVE as a cheap f16 2x TT
  - DVE (all f16, ~13us/tile): reverse scan max (~8.2us, irreducible),
    thr (tiny), w = (M[1:] >= thr)*t1h (STT 2x, accum -> S_k),
    z = lab_b*preD (TT 2x), w *= z (TT 2x)
  - M pad column preset once in two persistent buffers (no per-tile
    memset); ACT: sink Copy(w) accum -> A_k

Measured dead ends (same-process interleaved races): all-DMA-on-one-queue
(v0, +5us; each engine's HWDGE already spreads over 8 HW queues), SWDGE
vs HWDGE lab (±1us), io bufs=3 (±0), column-split DMA (±0), sub all-DVE
(+3us) or all-Pool (+2..6us: Pool->DVE handoff latency), scan-free
iota-max formulation (vn, +11us), z/sub on Pool in f16 (vo, +29us: the
Q7's f16 multiply is far slower than its f32 subtract), in-place d/z +
io bufs=3 (vs, +36us: in-place WAR serialization), deferring the ACT
sink one tile (vt, ±0), tensor_tensor_reduce in any form including the
qr.py dummy-broadcast pattern (crashes the device mesh).
"""

import os
from contextlib import ExitStack

import numpy as np

B, L = 4096, 4096
N_CORES = 8
ROWS_PER_CORE = B // N_CORES          # 512
P = 128                               # partitions per tile
TILES = ROWS_PER_CORE // P            # 4
C_CONST = 0.65 * 0.1                  # 0.065
BIG = 1e30

VARIANT = "vr"                        # kernel() uses this one

_CACHE = {}


def _build_nc(repeat: int = 1, variant: str = VARIANT):
    import concourse.mybir as mybir
    import concourse.tile as tile
    from concourse import bacc

    f32 = mybir.dt.float32
    f16 = mybir.dt.float16
    i32 = mybir.dt.int32
    Op = mybir.AluOpType
    Act = mybir.ActivationFunctionType

    # variant knobs
    #   ot_split: how ot's 4.19 MB/tile is routed over the two HWDGE queues
    #   lab_path: lab DMA queue + where the i32->f16 cast happens
    #   io_bufs:  io pool depth (DMA pipelining)
    #   use_ttr:  fuse w*z and the A-reduction into one DVE TTR (no ACT sink)
    knobs = {
        #         ot_split  lab_path  io_bufs  use_ttr  sub_eng
        "v0": ("sync",    "sync",   2, False, "pool"),
        "va": ("alt",     "alt",    2, False, "pool"),
        "vb": ("alt",     "swdge",  2, False, "pool"),
        "vd": ("col",     "swdge",  2, False, "pool"),
        "ve": ("alt",     "swdge",  3, False, "pool"),
        "vf": ("alt",     "swdge",  2, True,  "pool"),
        "vg": ("alt",     "swdge",  2, False, "act"),
        "vh": ("alt",     "swdge",  2, False, "split"),
        "vi": ("alt",     "alt",    2, False, "act"),
        "vj": ("alt",     "swdge",  2, False, "psplit"),
        "vk": ("alt",     "swdge",  2, False, "dve"),
        "vl": ("alt",     "alt",    2, False, "dve"),
        # vm = vj + lean: persistent M pad (no per-tile memset) and raw
        # acc_S/acc_A output with the C*S+A tail computed on host.
        "vm": ("alt",     "swdge",  2, False, "psplit"),
        # vn = scan-free: cut via m = max((j+1)*(d>=0)) fused in one STT;
        # d and z on Pool. Kills the 8.2us scan + memset + thr.
        "vn": ("alt",     "swdge",  2, False, "pool"),
        # vo = scan path, but sub AND z both on Pool: DVE keeps only
        # scan + STT + wz (+ tiny), ~14.6us/tile.
        "vo": ("alt",     "swdge",  2, False, "pool"),
        # vp/vq = all-f16 DVE: ACT casts t1 (and t0 for vp) to contiguous
        # f16 so the STT gets 2x; sub on DVE-f16 (vp) or Pool-f32 (vq).
        "vp": ("alt",     "swdge",  2, False, "dve"),
        "vq": ("alt",     "swdge",  2, False, "pool"),
        # vr = vp + persistent M buffers (pad memset once, not per tile).
        "vr": ("alt",     "swdge",  2, False, "dve"),
        # vs = vr + in-place d (into t0h) and z (into lab_b) + io bufs=3.
        "vs": ("alt",     "swdge",  3, False, "dve"),
        # vt = vr + sink deferred one tile, so ACT's casts for tile k+1
        # aren't queued behind sink(k) (which waits on wz(k)).
        "vt": ("alt",     "swdge",  2, False, "dve"),
        # vu = vr + wz fused with the A-accum in one dummy-broadcast TTR
        # on DVE (qr.py pattern); ACT does only the two casts.
        "vu": ("alt",     "swdge",  2, False, "dve"),
        # vw = vr + column-split ot DMA (both HWDGE queues fill each tile
        # simultaneously, halving the arrival latency that gates casts).
        "vw": ("col",     "swdge",  2, False, "dve"),
        # vx = vw + half-casts: each ACT cast covers half the columns and
        # starts as soon as its DMA half lands.
        "vx": ("col",     "swdge",  2, False, "dve"),
        # wa = vr + DVE queue reorder: z (depends only on the early SWDGE
        # lab DMA) is emitted before d, absorbing the ACT-cast latency.
        "wa": ("alt",     "swdge",  2, False, "dve"),
        # wb = vr minus the t0h cast: d = t0(strided f32) - t1h(f16) as a
        # mixed TT on DVE (1x). Rebalances ACT -7us/tile vs DVE +3us.
        "wb": ("alt",     "swdge",  2, False, "dve"),
        # wc = vr + in-place z (into lab_b) ALONE - untangles the vs
        # bundle and frees 16 KB SBUF.
        "wc": ("alt",     "swdge",  2, False, "dve"),
        # wd = wc + m_pool bufs=3: triple-buffered t1h/t0h casts hide the
        # per-tile sem latency in the DMA->cast->d->scan chain.
        "wd": ("alt",     "swdge",  2, False, "dve"),
        # we = vr + ACT sink writes a stride-0 dummy (frees 16 KB scratch)
        # + m_pool bufs=3 (triple-buffered casts) without in-place z.
        "we": ("alt",     "swdge",  2, False, "dve"),
    }
    probes = ("vz", "vy", "vz2", "pscan", "psub", "pstt", "ptt", "pred",
              "ppool", "pact", "pcast")
    if variant in probes:
        ot_split, lab_path, io_bufs, use_ttr, sub_eng = (
            "alt", "swdge", 2, False, "pool")
    else:
        ot_split, lab_path, io_bufs, use_ttr, sub_eng = knobs[variant]

    # Bacc (not raw Bass): its compile() runs generate_event_semaphores,
    # which splits multi-sem waits into standalone EventSemaphore
    # instructions (HW allows at most 1 wait per compute instruction).
    nc = bacc.Bacc("TRN2", target_bir_lowering=False, debug=False)

    out_d = nc.dram_tensor("out", [ROWS_PER_CORE, L * 2], f32, kind="ExternalInput")
    lab_d = nc.dram_tensor("lab", [ROWS_PER_CORE, L], i32, kind="ExternalInput")
    # pre holds two [P, L] constant planes: [:, 0:L] = preD, [:, L:2L] = j+1
    pre_d = nc.dram_tensor("pre", [P, L * 2], f32, kind="ExternalInput")
    res_d = nc.dram_tensor("res", [P, 1], f32, kind="ExternalOutput")

    out_t = out_d[:].rearrange("(n p) m -> n p m", p=P)   # [4, 128, 8192]
    lab_t = lab_d[:].rearrange("(n p) m -> n p m", p=P)   # [4, 128, 4096]

    with tile.TileContext(nc) as tc, ExitStack() as ctx:
        io_pool = ctx.enter_context(tc.tile_pool(name="io", bufs=io_bufs))
        pre_pool = ctx.enter_context(tc.tile_pool(name="pre", bufs=1))
        d_pool = ctx.enter_context(tc.tile_pool(name="d", bufs=2))
        m_pool = ctx.enter_context(
            tc.tile_pool(name="m", bufs=3 if variant in ("wd", "we")
                         else 2))
        w_pool = ctx.enter_context(tc.tile_pool(name="w", bufs=2))
        z_pool = ctx.enter_context(tc.tile_pool(name="z", bufs=2))
        acc_pool = ctx.enter_context(tc.tile_pool(name="acc", bufs=2))

        done = False
        # preamble (outside the repeat loop; excluded from per-iter time):
        # preD as f16 via SWDGE cast DMA, one-time.
        pre_b = pre_pool.tile([P, L], f16)
        if variant not in ("pscan", "psub", "pstt", "ptt", "pred", "ppool",
                           "pact"):
            nc.gpsimd.dma_start(pre_b[:], pre_d[:][:, 0:L])
        if variant == "vn":
            iota1 = pre_pool.tile([P, L], f32, tag="iota1")
            nc.sync.dma_start(iota1[:], pre_d[:][:, L:L * 2])
        if variant in ("vr", "vs", "vt", "vu", "vw", "vx", "wa", "wb",
                       "wc", "wd", "we"):
            M_a = pre_pool.tile([P, L + 1], f16, tag="Mpa")
            M_b = pre_pool.tile([P, L + 1], f16, tag="Mpb")
            M_pers = [M_a, M_b]
            nc.vector.memset(M_a[:, L:L + 1], -1.0)
            nc.vector.memset(M_b[:, L:L + 1], -1.0)

        if variant in ("pscan", "psub", "pstt", "ptt", "pred", "ppool",
                       "pact", "pcast"):
            pre_f = pre_pool.tile([P, L], f32, tag="pref")
            nc.sync.dma_start(pre_f[:], pre_d[:][:, 0:L])
            nc.scalar.activation(pre_b[:], pre_f[:], Act.Copy)
            # Engine-primitive probes: one op per repeat-iteration on
            # resident SBUF data (loaded once in the preamble). The
            # repeat-R-vs-repeat-r delta gives the op's true per-call cost.
            # Result is garbage by design.
            # All scratch allocated ONCE (bufs=1 pool); the loop rewrites
            # the same tiles from a single engine, so iterations serialize
            # in program order with no semaphores (pure op-rate measure).
            ot = pre_pool.tile([P, L * 2], f32, tag="pot")
            nc.sync.dma_start(ot[:], out_t[0])
            # single-HWDGE-queue preamble: the SP drain can only carry 4
            # sem waits, so probes avoid SWDGE/ACT queues entirely.
            lt0 = pre_pool.tile([P, L], i32, tag="plt0")
            nc.sync.dma_start(lt0[:], lab_t[0])
            lab_b = pre_pool.tile([P, L], f16, tag="plabb")
            nc.scalar.activation(lab_b[:], lt0[:], Act.Copy)
            x3 = ot[:].rearrange("p (l c) -> p l c", c=2)
            t0, t1 = x3[:, :, 0], x3[:, :, 1]
            d0 = pre_pool.tile([P, L], f16, tag="pd0")
            nc.vector.tensor_tensor(d0[:], t0, t1, Op.subtract)
            thr0 = pre_pool.tile([P, 1], f32, tag="pthr0")
            nc.vector.memset(thr0[:], 0.0)
            M = pre_pool.tile([P, L + 1], f16, tag="pM")
            nc.vector.memset(M[:, L:L + 1], -1.0)
            d = pre_pool.tile([P, L], f16, tag="pd")
            acc = pre_pool.tile([P, 1], f32, tag="pacc")
            w = pre_pool.tile([P, L], f16, tag="pw")
            z = pre_pool.tile([P, L], f16, tag="pz")
            mx = pre_pool.tile([P, 1], f32, tag="pmx")
            mxh = pre_pool.tile([P, 1], f16, tag="pmxh")
            loss_t = pre_pool.tile([P, 1], f32, tag="ploss")
            nc.vector.memset(loss_t[:], 0.0)
            for _r in range(repeat):
                if variant == "pscan":
                    nc.vector.tensor_tensor_scan(
                        M[:, 0:L][:, ::-1], d0[:, ::-1], d0[:, ::-1], -1.0,
                        Op.max, Op.max,
                    )
                    tgt = M
                elif variant == "psub":
                    nc.gpsimd.tensor_tensor(d[:], t0, t1, Op.subtract)
                    tgt = d
                elif variant == "pstt":
                    nc.vector.scalar_tensor_tensor(
                        w[:], d0[:], thr0[:], t1,
                        Op.is_ge, Op.mult,
                        accum_out=acc[:],
                    )
                    tgt = w
                elif variant == "ptt":
                    nc.vector.tensor_tensor(z[:], lab_b[:], pre_b[:], Op.mult)
                    tgt = z
                elif variant == "pred":
                    nc.vector.reduce_max(mx[:], d0[:], axis=mybir.AxisListType.X)
                    tgt = mx
                elif variant == "ppool":
                    # InstPool max on DVE (vs pred's tensor_reduce max)
                    nc.vector.pool_max(mxh[:], d0[:])
                    tgt = mxh
                elif variant == "pcast":
                    # ACT cast from STRIDED f32 (t1) -> contiguous f16:
                    # the op the vr pipeline runs twice per tile.
                    nc.scalar.activation(w[:], t1, Act.Copy)
                    tgt = w
                else:  # pact: ACT sink copy with accum
                    nc.scalar.activation(
                        w[:], d0[:], Act.Copy,
                        accum_out=acc[:],
                    )
                    tgt = w
                # cheap consumer: keeps every write consumed so the final
                # Drain doesn't accumulate unbounded sem waits.
                nc.vector.tensor_tensor(
                    loss_t[:], loss_t[:], tgt[:, 0:1], Op.add)
            nc.sync.dma_start(res_d[:], loss_t[:])
            done = True

        if variant in ("vz", "vy", "vz2"):
            # DMA-only probe: measures the pure HBM streaming floor.
            # vz: balanced SP/ACT HWDGE queues (12.6 MB each); vy: all on
            # SP; vz2: the vj layout (alt ot + SWDGE cast-DMA lab).
            # Result is garbage (zeros) by design.
            loss_t = acc_pool.tile([P, 1], f32, tag="loss")
            nc.vector.memset(loss_t[:], 0.0)
            for _r in range(repeat):
                for k in range(TILES):
                    ot = io_pool.tile([P, L * 2], f32, tag="ot")
                    if variant == "vy":
                        nc.sync.dma_start(ot[:], out_t[k])
                        lt = io_pool.tile([P, L], i32, tag="lt")
                        nc.sync.dma_start(lt[:], lab_t[k])
                    elif variant == "vz2":
                        ot_e = nc.sync if k % 2 == 0 else nc.scalar
                        ot_e.dma_start(ot[:], out_t[k])
                        lab_b = z_pool.tile([P, L], f16, tag="labb")
                        nc.gpsimd.dma_start(lab_b[:], lab_t[k])
                    else:
                        ot_e = nc.sync if k % 2 == 0 else nc.scalar
                        lab_e = nc.scalar if k % 2 == 0 else nc.sync
                        ot_e.dma_start(ot[:], out_t[k])
                        lt = io_pool.tile([P, L], i32, tag="lt")
                        lab_e.dma_start(lt[:], lab_t[k])
            nc.sync.dma_start(res_d[:], loss_t[:])
            done = True

        for _r in (range(repeat) if not done else ()):
            acc_S = acc_pool.tile([P, TILES], f32, tag="accS")
            acc_A = acc_pool.tile([P, TILES], f32, tag="accA")
            pend = []
            for k in range(TILES):
                ot = io_pool.tile([P, L * 2], f32, tag="ot")
                if ot_split == "sync":
                    nc.sync.dma_start(ot[:], out_t[k])
                elif ot_split == "alt":
                    eng = nc.sync if k % 2 == 0 else nc.scalar
                    eng.dma_start(ot[:], out_t[k])
                else:  # col: half the columns per HWDGE queue, every tile
                    nc.sync.dma_start(ot[:, 0:L], out_t[k][:, 0:L])
                    nc.scalar.dma_start(ot[:, L:L * 2], out_t[k][:, L:L * 2])

                if lab_path == "swdge":
                    # SWDGE cast DMA: reads i32 from HBM, writes f16 SBUF.
                    lab_b = z_pool.tile([P, L], f16, tag="labb")
                    nc.gpsimd.dma_start(lab_b[:], lab_t[k])
                else:
                    lab_eng = nc.sync if (
                        lab_path == "sync" or k % 2 == 1) else nc.scalar
                    lt = io_pool.tile([P, L], i32, tag="lt")
                    lab_eng.dma_start(lt[:], lab_t[k])
                    # ACT: int32 -> f16 cast on-engine.
                    lab_b = z_pool.tile([P, L], f16, tag="labb")
                    nc.scalar.activation(lab_b[:], lt[:], Act.Copy)

                x3 = ot[:].rearrange("p (l c) -> p l c", c=2)
                t0 = x3[:, :, 0]
                t1 = x3[:, :, 1]

                if variant in ("vp", "vq", "vr", "vs", "vt", "vu", "vw",
                               "vx", "wa", "wb", "wc", "wd", "we"):
                    if variant == "wa":
                        # z first in DVE's queue: it depends only on the
                        # early SWDGE lab DMA, so DVE does useful work
                        # while ACT is still casting t0/t1.
                        z = z_pool.tile([P, L], f16, tag="z")
                        nc.vector.tensor_tensor(z[:], lab_b[:], pre_b[:],
                                                Op.mult)
                    # ACT: t1 -> contiguous f16 (makes the STT 2x).
                    t1h = m_pool.tile([P, L], f16, tag="t1h")
                    if variant == "vx":
                        h = L // 2
                        nc.scalar.activation(
                            t1h[:, 0:h], x3[:, 0:h, 1], Act.Copy)
                        nc.scalar.activation(
                            t1h[:, h:L], x3[:, h:L, 1], Act.Copy)
                    else:
                        nc.scalar.activation(t1h[:], t1, Act.Copy)
                    if variant == "wb":
                        # mixed TT: strided f32 t0 minus f16 t1h (1x).
                        d = d_pool.tile([P, L], f16)
                        nc.vector.tensor_tensor(d[:], t0, t1h[:],
                                                Op.subtract)
                    elif variant == "vq":
                        # vq: d on Pool from the raw strided f32.
                        d = d_pool.tile([P, L], f16)
                        nc.gpsimd.tensor_tensor(d[:], t0, t1, Op.subtract)
                    else:
                        # ACT: t0 -> f16 too; d on DVE (TT f16 2x).
                        t0h = m_pool.tile([P, L], f16, tag="t0h")
                        if variant == "vx":
                            h = L // 2
                            nc.scalar.activation(
                                t0h[:, 0:h], x3[:, 0:h, 0], Act.Copy)
                            nc.scalar.activation(
                                t0h[:, h:L], x3[:, h:L, 0], Act.Copy)
                        else:
                            nc.scalar.activation(t0h[:], t0, Act.Copy)
                        if variant == "vs":
                            # in-place: d overwrites t0h (saves a pool)
                            d = t0h
                            nc.vector.tensor_tensor(
                                d[:], t0h[:], t1h[:], Op.subtract)
                        else:
                            d = d_pool.tile([P, L], f16)
                            nc.vector.tensor_tensor(
                                d[:], t0h[:], t1h[:], Op.subtract)

                    if variant in ("vr", "vs", "vt", "vu", "vw", "vx",
                                   "wa", "wb", "wc", "wd", "we"):
                        M = M_pers[k % 2]
                    else:
                        M = w_pool.tile([P, L + 1], f16, tag="M")
                        nc.vector.memset(M[:, L:L + 1], -1.0)
                    nc.vector.tensor_tensor_scan(
                        M[:, 0:L][:, ::-1], d[:, ::-1], d[:, ::-1], -1.0,
                        Op.max, Op.max,
                    )
                    thr = acc_pool.tile([P, 1], f32, tag="thr")
                    nc.vector.tensor_scalar(
                        thr[:], M[:, 0:1], 0.0, -BIG, Op.is_lt, Op.mult
                    )
                    # DVE: w = (M[j+1] >= thr) * t1h -- all f16, 2x.
                    w = w_pool.tile([P, L], f16)
                    nc.vector.scalar_tensor_tensor(
                        w[:], M[:, 1:L + 1], thr[:], t1h[:],
                        Op.is_ge, Op.mult,
                        accum_out=acc_S[:, k:k + 1],
                    )
                    if variant in ("vs", "wc", "wd"):
                        # in-place: z overwrites lab_b (saves a pool)
                        z = lab_b
                        nc.vector.tensor_tensor(z[:], lab_b[:], pre_b[:],
                                                Op.mult)
                    elif variant != "wa":  # wa emitted z before the casts
                        z = z_pool.tile([P, L], f16, tag="z")
                        nc.vector.tensor_tensor(z[:], lab_b[:], pre_b[:],
                                                Op.mult)
                    if variant == "vu":
                        # fused wz + A-accum: dummy broadcast out (stride
                        # 0), accum_out carries the real result.
                        dum = acc_pool.tile([P, 1], f16, tag="dumA")
                        nc.vector.tensor_tensor_reduce(
                            dum.broadcast_to((P, L)), w[:], z[:],
                            1.0, 0.0, Op.mult, Op.add,
                            accum_out=acc_A[:, k:k + 1],
                        )
                        continue
                    nc.vector.tensor_tensor(w[:], w[:], z[:], Op.mult)
                    if variant == "vt":
                        # defer sink(k) until after tile k+1's ACT casts
                        pend.append((w, k))
                        if len(pend) > 1:
                            w_p, k_p = pend.pop(0)
                            sink = w_pool.tile([P, L], f16, tag="sink")
                            nc.scalar.activation(
                                sink[:], w_p[:], Act.Copy,
                                accum_out=acc_A[:, k_p:k_p + 1],
                            )
                        continue
                    if variant == "we":
                        # stride-0 dummy out: only the accumulator is real
                        dums = acc_pool.tile([P, 1], f16, tag="dumS")
                        nc.scalar.activation(
                            dums.broadcast_to((P, L)), w[:], Act.Copy,
                            accum_out=acc_A[:, k:k + 1],
                        )
                        continue
                    sink = w_pool.tile([P, L], f16, tag="sink")
                    nc.scalar.activation(
                        sink[:], w[:], Act.Copy,
                        accum_out=acc_A[:, k:k + 1],
                    )
                    continue

                if variant == "vn":
                    # scan-free mask: d = t0-t1 (Pool); y = (d>=0)*(j+1)
                    # in ONE DVE STT; m = max_j y = 1 + (last j with d>=0),
                    # or 0 if none; mask[j] = (j+1 < m'), m' = m (or L+1
                    # when m==0).  z = lab*preD on Pool (Q7 mult).
                    d = d_pool.tile([P, L], f16)
                    nc.gpsimd.tensor_tensor(d[:], t0, t1, Op.subtract)
                    # z on DVE (f16 2x) — the Q7's f16 multiply is slow
                    z = z_pool.tile([P, L], f16, tag="z")
                    nc.vector.tensor_tensor(z[:], lab_b[:], pre_b[:],
                                            Op.mult)
                    y = m_pool.tile([P, L], f32, tag="y")
                    nc.vector.scalar_tensor_tensor(
                        y[:], d[:], 0.0, iota1[:],
                        Op.is_ge, Op.mult,
                    )
                    mq = acc_pool.tile([P, 1], f32, tag="mq")
                    nc.vector.reduce_max(mq[:], y[:],
                                         axis=mybir.AxisListType.X)
                    # m' = m + (m==0)*(L+1), two tiny [P,1] ops
                    eq2 = acc_pool.tile([P, 1], f32, tag="eq2")
                    nc.vector.tensor_scalar(
                        eq2[:], mq[:], 0.0, float(L + 1), Op.is_equal,
                        Op.mult)
                    nc.vector.tensor_tensor(mq[:], mq[:], eq2[:], Op.add)
                    # w = (iota1 < m') * t1, S_k = sum(w)
                    w = w_pool.tile([P, L], f16)
                    nc.vector.scalar_tensor_tensor(
                        w[:], iota1[:], mq[:], t1,
                        Op.is_lt, Op.mult,
                        accum_out=acc_S[:, k:k + 1],
                    )
                    # w *= z (TT f16, 2x, in-place)
                    nc.vector.tensor_tensor(w[:], w[:], z[:], Op.mult)
                    # ACT: sink copy with accumulator -> A_k
                    sink = w_pool.tile([P, L], f16, tag="sink")
                    nc.scalar.activation(
                        sink[:], w[:], Act.Copy,
                        accum_out=acc_A[:, k:k + 1],
                    )
                    continue

                # d = t0 - t1 (f16 out).  Pool's Q7 software TT is slow on
                # strided f32 reads, so variants move this to ACT.
                d = d_pool.tile([P, L], f16)
                if sub_eng == "pool":
                    nc.gpsimd.tensor_tensor(d[:], t0, t1, Op.subtract)
                elif sub_eng == "dve":
                    nc.vector.tensor_tensor(d[:], t0, t1, Op.subtract)
                elif sub_eng == "psplit":
                    # low 3/8 on Pool (its Q7 sub is ~4x slower per elem),
                    # high 5/8 on DVE, so both finish together.
                    h = 3 * L // 8
                    nc.gpsimd.tensor_tensor(
                        d[:, 0:h], t0[:, 0:h], t1[:, 0:h], Op.subtract)
                    nc.vector.tensor_tensor(
                        d[:, h:L], t0[:, h:L], t1[:, h:L], Op.subtract)
                elif sub_eng == "act":
                    nc.scalar.tensor_tensor(d[:], t0, t1, Op.subtract)
                else:  # split: low half on Pool, high half on ACT
                    h = L // 2
                    nc.gpsimd.tensor_tensor(
                        d[:, 0:h], t0[:, 0:h], t1[:, 0:h], Op.subtract)
                    nc.scalar.tensor_tensor(
                        d[:, h:L], t0[:, h:L], t1[:, h:L], Op.subtract)

                # DVE scan: M[j] = max(d[j:], -1), M[L] = -1 pad (f16).
                # Ordered before z so ACT's sink(k-1) hides under the
                # scan+STT window instead of stalling DVE's queue head.
                M = m_pool.tile([P, L + 1], f16)
                nc.vector.memset(M[:, L:L + 1], -1.0)
                nc.vector.tensor_tensor_scan(
                    M[:, 0:L][:, ::-1], d[:, ::-1], d[:, ::-1], -1.0,
                    Op.max, Op.max,
                )

                # tiny (DVE): thr = 0 if M[0] >= 0 else -BIG, one fused TS:
                # (M0 < 0) * -BIG
                thr = acc_pool.tile([P, 1], f32, tag="thr")
                nc.vector.tensor_scalar(
                    thr[:], M[:, 0:1], 0.0, -BIG, Op.is_lt, Op.mult
                )

                # DVE: w = (M[j+1] >= thr) * t1 (f16 out), S_k = sum(w).
                w = w_pool.tile([P, L], f16)
                nc.vector.scalar_tensor_tensor(
                    w[:], M[:, 1:L + 1], thr[:], t1,
                    Op.is_ge, Op.mult,
                    accum_out=acc_S[:, k:k + 1],
                )

                # z = lab_b * preD (TT f16): on Pool for vo (frees DVE),
                # else DVE (2x), late on purpose.
                z = z_pool.tile([P, L], f16, tag="z")
                z_eng = nc.gpsimd if variant == "vo" else nc.vector
                z_eng.tensor_tensor(z[:], lab_b[:], pre_b[:], Op.mult)

                if use_ttr:
                    # DVE: w = w*z fused with A_k = sum(w*z); no ACT sink.
                    nc.vector.tensor_tensor_reduce(
                        w[:], w[:], z[:], 1.0, 0.0, Op.mult, Op.add,
                        accum_out=acc_A[:, k:k + 1],
                    )
                else:
                    # DVE: w *= z (TT f16, 2x, in-place).
                    nc.vector.tensor_tensor(w[:], w[:], z[:], Op.mult)

                    # ACT: sink copy with accumulator -> A_k = sum(w*z).
                    sink = w_pool.tile([P, L], f16, tag="sink")
                    nc.scalar.activation(
                        sink[:], w[:], Act.Copy,
                        accum_out=acc_A[:, k:k + 1],
                    )

            for w_p, k_p in pend:
                sink = w_pool.tile([P, L], f16, tag="sink")
                nc.scalar.activation(
                    sink[:], w_p[:], Act.Copy,
                    accum_out=acc_A[:, k_p:k_p + 1],
                )

            if k == TILES - 1:
                # tail: loss_i = C*sum_k S_k + sum_k A_k
                t4 = acc_pool.tile([P, TILES], f32, tag="t4")
                nc.vector.tensor_scalar(t4[:], acc_S[:], C_CONST, None, Op.mult)
                nc.vector.tensor_tensor(t4[:], t4[:], acc_A[:], Op.add)
                loss_t = acc_pool.tile([P, 1], f32, tag="loss")
                nc.vector.reduce_sum(loss_t[:], t4[:], axis=mybir.AxisListType.X)

        if not done:
            nc.sync.dma_start(res_d[:], loss_t[:])

    nc.compile()
    return nc


def _pre_tile() -> np.ndarray:
    j = np.arange(L, dtype=np.float64)
    pre2 = (-3.6 / np.log2(j + 2.0) - C_CONST).astype(np.float32)
    iota1 = (j + 1.0).astype(np.float32)
    plane = np.concatenate([pre2, iota1])
    return np.ascontiguousarray(np.tile(plane[None, :], (P, 1)))


def _get_nc(repeat: int = 1, variant: str = VARIANT):
    key = (repeat, variant)
    if key not in _CACHE:
        _CACHE[key] = _build_nc(repeat=repeat, variant=variant)
    return _CACHE[key]


def make_in_maps(output: np.ndarray, labels: np.ndarray):
    pre = _pre_tile()
    in_maps = []
    for c in range(N_CORES):
        sl = slice(c * ROWS_PER_CORE, (c + 1) * ROWS_PER_CORE)
        in_maps.append({
            "out": np.ascontiguousarray(output[sl]).reshape(ROWS_PER_CORE, L * 2),
            "lab": np.ascontiguousarray(labels[sl]),
            "pre": pre,
        })
    return in_maps


def kernel(output: np.ndarray, labels: np.ndarray) -> np.ndarray:
    from concourse.bass_utils import run_bass_kernel_spmd

    nc = _get_nc(repeat=1)
    in_maps = make_in_maps(output, labels)
    r = run_bass_kernel_spmd(nc, in_maps, core_ids=list(range(N_CORES)))
    total = 0.0
    for res in r.results:
        total += float(res["res"].astype(np.float64).sum())
    return np.float32(total / B)


if __name__ == "__main__":
    # quick standalone run (full inputs, random)
    rng = np.random.default_rng(0)
    out = rng.standard_normal((B, L, 2)).astype(np.float32)
    lab = rng.integers(0, 2, size=(B, L)).astype(np.int32)
    print("loss:", kernel(out, lab))
